# revision 30
# baseline (speedup 1.0000x reference)
"""BNAF forward (B=2048, D=8, H=512, 4 masked layers) on 8 TRN2 NeuronCores.

Strategy
--------
Pure data parallel: batch is split 256/core; the small weights are replicated.

Math: the BNAF log-det recursion collapses in exp space.  For each masked
linear layer, exp(logdet diag blocks) == the diag blocks of the normalized
weight w itself, and for tanh, exp(logdet) == 1 - h^2.  So the whole
log-sum-exp flow is a chain of *positive* block-diagonal matmuls with one
log() at the very end.  The per-output norm scale s = exp(logg)/||v|| is
folded into the G-flow moving operand (G_in = s * G), so the G-flow
stationary is just exp(W) block-diag.

v2 schedule notes (on top of the v1 design):
- the ACT queue order is pinned explicitly (esm exp -> l2 diag exps ->
  l1 tanhs -> l3 diag exps -> l2 tanhs -> l3 tanhs -> l4 tanh) with
  order-only dep edges, so layer-l tanhs are never stuck behind layer-l+1
  prep in the in-order ACT queue.
- vsq (norm squares) moved from ACT to DVE (one fp16 tensor_tensor per
  layer over the whole packed strip).
- the layer-1 stationary is built by exp-ing the natural [128,32] masked
  layout (part of the single batched esm exp) and PE-transposing it into
  [8,512]; this kills the slow 8-partition [8,512] EXP+ADD chain and the
  separate w1s DMA.
- bias4 is injected into the z4/PSUM accumulation with a K=1 ones-row
  matmul; the final tail is 2 fused stt ops + fast-log.
- smalls DMA split so the exp block lands first; weight cast-DMAs
  reordered (vt2 diag first), xT moved to the scalar queue.
- ACT only ever uses {Exp, Tanh}: single table load at start.
"""

import numpy as np

TRACE = False          # set by test.py for profiling runs
LAST_RESULTS = None    # BassKernelResults stash for test.py

_CACHE = {}

P = 128
BC = 256          # batch per core
H = 512
NCORE = 8
MAGIC = 0x5f3759df
OFF2 = (512, 896, 1152)   # packed col offsets of the strictly-lower windows
LN2_A = 8.262958294867817e-08     # ln2 * 2^-23
LN2_B = -90.77247532458875        # -126.9570 * ln2 - 4*ln2 (G-flow 2^4 scale)

# smalls layout: first the exp block (exp'd in one ACT op), then the rest,
# then the l2/l3 diag strips (exp-able quadrants + raw UR quadrants) so the
# whole latency-critical small-weight path rides ONE fast HWDGE DMA.
# wNmd* entries hold where(mask_d, W, -100): exp gives exp(W)*mask_d exactly,
# so the masked-linear weights need no on-device mask multiplies.
_SM = {}
_off = 0
for _name, _w in [("w1dg", 4), ("lg1", 4), ("lg2", 4), ("lg3", 4),
                  ("lg4r", 8), ("w1mdN", 32), ("w4mdT", 32),      # exp block
                  ("b1", 4), ("b2", 4), ("b3", 4), ("b4rep", 16),
                  ("w1moN", 32), ("w4moT", 32)]:
    _SM[_name] = (_off, _off + _w)
    _off += _w
SMALL_W = _off
EXPW = _SM["w4mdT"][1]        # width of the exp block (88)


def _vsl(vt, k, c):
    """Packed-layout slice of the (in-chunk k, out-chunk c) 128x128 block."""
    if k == c:
        return vt[:, 128 * k:128 * k + 128]
    o = OFF2[k] + 128 * (c - k - 1)
    return vt[:, o:o + 128]


def _build():
    import concourse.bacc as bacc
    import concourse.mybir as mybir
    import concourse.tile as tile
    from concourse.tile_rust import add_dep_helper
    from concourse.masks import make_identity
    from contextlib import ExitStack

    f32 = mybir.dt.float32
    u32 = mybir.dt.uint32
    fp16 = mybir.dt.float16
    E = mybir.ActivationFunctionType
    ALU = mybir.AluOpType

    nc = bacc.Bacc("TRN2", target_bir_lowering=False, debug=False,
                   enable_asserts=False, num_devices=NCORE)

    t = {}
    t["xT4"] = nc.dram_tensor("xT4", (P, BC), f32, kind="ExternalInput").ap()
    t["wp2d"] = nc.dram_tensor("wp2d", (P, H), f32, kind="ExternalInput").ap()
    t["wp2w"] = nc.dram_tensor("wp2w", (P, 768), f32, kind="ExternalInput").ap()
    t["wp3d"] = nc.dram_tensor("wp3d", (P, H), f32, kind="ExternalInput").ap()
    t["wp3w"] = nc.dram_tensor("wp3w", (P, 768), f32, kind="ExternalInput").ap()
    t["smalls"] = nc.dram_tensor("smalls", (P, SMALL_W), f32, kind="ExternalInput").ap()
    t["h4T_out"] = nc.dram_tensor("h4T_out", (P, 16), f32, kind="ExternalOutput").ap()
    t["sldT_out"] = nc.dram_tensor("sldT_out", (P, 16), f32, kind="ExternalOutput").ap()

    def mm(out, lhsT, rhs, **kw):
        return nc.tensor.matmul(out, lhsT, rhs, **kw)

    def dep(a, b):
        """Pin engine-queue order: instruction a runs before b."""
        add_dep_helper(b.ins, a.ins, False, "act-order")

    with tile.TileContext(nc) as tc, ExitStack() as ctx:
        wgt = ctx.enter_context(tc.tile_pool(name="wgt", bufs=1))
        scr = ctx.enter_context(tc.tile_pool(name="scr", bufs=3))
        pz = ctx.enter_context(tc.tile_pool(name="pz", bufs=1, space="PSUM"))
        pf = ctx.enter_context(tc.tile_pool(name="pf", bufs=1, space="PSUM"))
        pn = ctx.enter_context(tc.tile_pool(name="pn", bufs=1, space="PSUM"))
        ptr = ctx.enter_context(tc.tile_pool(name="ptr", bufs=1, space="PSUM"))

        act = nc.scalar.activation
        cp = nc.vector.tensor_copy
        ts = nc.vector.tensor_scalar
        stt = nc.vector.scalar_tensor_tensor
        mul = nc.vector.tensor_mul
        tt = nc.vector.tensor_tensor

        # ---- input DMAs ----
        # smalls on the fast first HWDGE DMA (sem ~9.2us); x (row-replicated
        # 4x for the 32-aligned L1 stationary slices) on the scalar queue
        # (sem ~9.6us); the packed weight strips as gpsimd fp32->fp16
        # cast-DMAs (sems ~11.2us + ~0.6/queue-slot).
        smalls = wgt.tile([P, SMALL_W], f32, name="smalls_t", tag="smalls_t")
        nc.sync.dma_start(smalls, t["smalls"])
        xT4 = wgt.tile([P, BC], f32, name="xT4", tag="xT4")
        nc.scalar.dma_start(xT4, t["xT4"])
        # gpsimd queue: tiny memsets + identity first, then the weight DMAs
        wz = wgt.tile([P, BC], fp16, name="wz", tag="wz")
        nc.gpsimd.memset(wz, 0.0)
        ident = wgt.tile([P, P], f32, name="ident", tag="ident")
        make_identity(nc, ident)
        vt2 = wgt.tile([P, 1280], fp16, name="vt2", tag="vt2")
        vt3 = wgt.tile([P, 1280], fp16, name="vt3", tag="vt3")
        nc.gpsimd.dma_start(vt2[:, 0:H], t["wp2d"])
        nc.gpsimd.dma_start(vt2[:, H:1280], t["wp2w"])
        nc.gpsimd.dma_start(vt3[:, 0:H], t["wp3d"])
        nc.gpsimd.dma_start(vt3[:, H:1280], t["wp3w"])

        def sm(name):
            a, b = _SM[name]
            return smalls[:, a:b]

        # ---- tiny constants on DVE (keep Q7 free) ----
        magict = wgt.tile([P, 8], u32, name="magict", tag="magict")
        nc.vector.memset(magict, MAGIC)
        ones4f = wgt.tile([P, 4], f32, name="ones4f", tag="ones4f")
        nc.vector.memset(ones4f, 1.0)
        ones4 = wgt.tile([P, 4], fp16, name="ones4", tag="ones4")
        cp(ones4, ones4f)
        onesr = wgt.tile([1, P], f32, name="onesr", tag="onesr")
        nc.vector.memset(onesr, 1.0)

        # short PE warm-up burst (HAM un-throttle) while DMAs drain
        pw = pn.tile([2, BC - 2], f32, name="pw", tag="pn")
        for _ in range(12):
            mm(pw, wz[:, 0:2], wz[:, 2:BC], skip_group_check=True)

        # one batched exp over the whole exp block
        esm = wgt.tile([P, EXPW], f32, name="esm", tag="esm")
        A_esm = act(esm, smalls[:, 0:EXPW], E.Exp)

        def esl(name):
            a, b = _SM[name]
            return esm[:, a:b]

        e1d = esl("w1dg")
        eg = {1: esl("lg1"), 2: esl("lg2"), 3: esl("lg3")}
        eg4row = esm[0:1, _SM["lg4r"][0]:_SM["lg4r"][0] + 8]
        e1mdN = esl("w1mdN")
        e4mdT = esl("w4mdT")

        # s = eg * rsqrt(norm2): DVE-only Newton rsqrt (reads n2 psum directly)
        def make_scale(n2_ap, eg_ap, shape, nm):
            pr = shape[0]
            shf = scr.tile(list(shape), u32, name=f"shf_{nm}", tag="sc_shf")
            ts(shf, n2_ap.bitcast(u32), 1, None, op0=ALU.arith_shift_right)
            y0 = scr.tile(list(shape), u32, name=f"y0_{nm}", tag="sc_y0")
            stt(y0, magict[:pr, :shape[1]], 0, shf, op0=ALU.bypass, op1=ALU.subtract)
            y = y0.bitcast(f32)
            t1 = scr.tile(list(shape), f32, name=f"t1_{nm}", tag="sc_t1")
            t2 = scr.tile(list(shape), f32, name=f"t2_{nm}", tag="sc_t2")
            for it in range(1):         # one Newton step: y *= 1.5 - 0.5*n2*y*y
                mul(t1, y, y)
                mul(t2, t1, n2_ap)
                ts(t1, t2, -0.5, 1.5, op0=ALU.mult, op1=ALU.add)
                yn = scr.tile(list(shape), f32, name=f"yn{it}_{nm}", tag=f"sc_yn{it}")
                mul(yn, y, t1)
                y = yn
            s = wgt.tile(list(shape), f32, name=f"s_{nm}", tag=f"s_{nm}")
            mul(s, eg_ap, y)
            return s

        # ================= layer 1 prep ===================================
        # v1n in a 32-col-padded natural layout (chunk c at cols 32c:32c+8):
        # one strided add, ONE PE transpose, one copy.  The L1 stationary is
        # then v1TP[32c:32c+8, :] (32-aligned partition slices, matching the
        # row-replicated xT4 moving operand), consumed as f32r.
        with tc.high_priority():
            v1nP = wgt.tile([P, P], f32, name="v1nP", tag="v1nP")
            v1nv = v1nP.rearrange("p (b c) -> p b c", c=32)[:, :, 0:8]
            tt(v1nv, e1mdN.rearrange("p (b c) -> p b c", c=8),
               sm("w1moN").rearrange("p (b c) -> p b c", c=8), op=ALU.add)
            n1 = wgt.tile([P, 4], f32, name="n1", tag="n1")
            for c in range(4):
                sq1 = scr.tile([P, 8], f32, name=f"sq1_{c}", tag="sq1")
                stt(sq1, v1nP[:, 32 * c:32 * c + 8], 0, v1nP[:, 32 * c:32 * c + 8],
                    op0=ALU.bypass, op1=ALU.mult, accum_out=n1[:, c:c + 1])
            s1 = make_scale(n1, eg[1], (P, 4), "l1")
            e1s = wgt.tile([P, 4], f32, name="e1s", tag="e1s")
            stt(e1s, e1d, 16.0, s1, op0=ALU.mult, op1=ALU.mult)
            # chunk 3 would land at base partition 96 (invalid: bases must be
            # 0/32/64), so it gets its own base-0 transpose of the last slice
            ptp = ptr.tile([P, P], f32, name="ptp", tag="ptr")
            nc.tensor.transpose(ptp, v1nP, ident)
            v1TP = wgt.tile([P, P], fp16, name="v1TP", tag="v1TP")
            cp(v1TP, ptp)
            ptp2 = ptr.tile([32, P], f32, name="ptp2", tag="ptr")
            nc.tensor.transpose(ptp2, v1nP[:, 96:128], ident)
            v1Tb = wgt.tile([8, P], fp16, name="v1Tb", tag="v1Tb")
            cp(v1Tb, ptp2[0:8, :])
            xT4h = wgt.tile([P, BC], fp16, name="xT4h", tag="xT4h")
            cp(xT4h, xT4)

        # ======= layer 4 prep (cheap; all inputs ready early, so keep the
        # whole chain high-priority to stop the scheduler drifting it late) ==
        with tc.high_priority():
            vt4 = wgt.tile([P, 32], fp16, name="vt4", tag="vt4")
            tt(vt4, e4mdT, sm("w4moT"), op=ALU.add)
            vsq4 = scr.tile([P, 32], fp16, name="vsq4", tag="vsq4")
            mul(vsq4, vt4, vt4)
            n4 = pn.tile([1, 8], f32, name="n4", tag="pn")
            for k in range(4):
                mm(n4, ones4[:, 0:1], vsq4[:, 8 * k:8 * k + 8],
                   start=(k == 0), stop=(k == 3))
            s4r = make_scale(n4, eg4row, (1, 8), "l4")
            # broadcast s4 to all partitions with a K=1 ones-row matmul
            s4x = pn.tile([P, 32], f32, name="s4x", tag="pn")
            for k in range(4):
                mm(s4x[:, 8 * k:8 * k + 8], onesr, s4r)
            # fold s4 into both layer-4 stationaries (scale along out-feats)
            vt4s = wgt.tile([P, 32], fp16, name="vt4s", tag="vt4s")
            mul(vt4s, vt4, s4x)
            vd4s = wgt.tile([P, 32], fp16, name="vd4s", tag="vd4s")
            mul(vd4s, e4mdT, s4x)

        # ================= layer 1 batch ==================================
        h1 = wgt.tile([P, 4 * BC], fp16, name="h1", tag="h1")
        G1 = wgt.tile([P, 4 * BC], fp16, name="G1", tag="G1")
        T1 = []
        for c in range(4):
            cs = slice(BC * c, BC * c + BC)
            zc = pz.tile([P, BC], f32, name=f"pz1_{c}", tag=f"pzc{c}")
            if c < 3:
                mm(zc, v1TP[32 * c:32 * c + 8, :], xT4h[32 * c:32 * c + 8, :])
            else:
                mm(zc, v1Tb[0:8, :], xT4h[0:8, :])
            T1.append(act(h1[:, cs], zc, E.Tanh,
                          bias=sm("b1")[:, c:c + 1], scale=s1[:, c:c + 1]))
            # G1 = (1 - h^2) * e1s: 2 fused DVE ops per chunk
            nh2 = scr.tile([P, BC], fp16, name=f"nh1_{c}", tag="hqc")
            stt(nh2, h1[:, cs], -1.0, h1[:, cs], op0=ALU.mult, op1=ALU.mult)
            ts(G1[:, cs], nh2, 1.0, e1s[:, c:c + 1], op0=ALU.add, op1=ALU.mult)

        # ================= layer 2/3 prep ================================
        def prep_stat(l, vt):
            # in-place exp of the 8 diag 64x64 blocks: two strided ACT ops
            dA = vt[0:64, 0:H].rearrange("p (b c) -> p b c", c=128)[:, :, 0:64]
            a1 = act(dA, dA, E.Exp)
            dB = vt[64:128, 0:H].rearrange("p (b c) -> p b c", c=128)[:, :, 64:128]
            a2 = act(dB, dB, E.Exp)
            # vsq on DVE, split so the window part (gated only by its DMA)
            # can run before the diag part (gated by the exps)
            vsq = scr.tile([P, 1280], fp16, name=f"vsq{l}", tag="vsq")
            mul(vsq[:, H:1280], vt[:, H:1280], vt[:, H:1280])
            mul(vsq[:, 0:H], vt[:, 0:H], vt[:, 0:H])
            return vsq, a1, a2

        def prep_norms(l, vsq):
            # norm2 directly in partition layout: vsq-stationary x ones-col
            ncol = pn.tile([P, 4], f32, name=f"ncol{l}", tag="pn")
            with tc.high_priority():
                for c in range(4):
                    for k in range(c + 1):
                        mm(ncol[:, c:c + 1], _vsl(vsq, k, c), ones4[:, 0:1],
                           start=(k == 0), stop=(k == c))
                return make_scale(ncol, eg[l], (P, 4), f"l{l}")

        vsq2, A_dA2, A_dB2 = prep_stat(2, vt2)
        s2 = prep_norms(2, vsq2)

        # ================= layer 2/3 batch ================================
        def big_batch(l, vt, s, h_prev, G_prev):
            hl = wgt.tile([P, 4 * BC], fp16, name=f"h{l}", tag=f"h{l}")
            Gl = wgt.tile([P, 4 * BC], fp16, name=f"G{l}", tag=f"G{l}")
            tanhs = []
            for c in range(4):
                cs = slice(BC * c, BC * c + BC)
                zc = pz.tile([P, BC], f32, name=f"pz{l}_{c}", tag=f"pzc{c}")
                for k in range(c + 1):
                    mm(zc, _vsl(vt, k, c), h_prev[:, BC * k:BC * k + BC],
                       start=(k == 0), stop=(k == c))
                tanhs.append(act(hl[:, cs], zc, E.Tanh,
                                 bias=sm(f"b{l}")[:, c:c + 1], scale=s[:, c:c + 1]))
                # pair (c0,c3) and (c1,c2): c3's buffer is freed by c0's
                # early Gin read, so the late G-tail never stalls on PSUM
                fc = pf.tile([P, BC], f32, name=f"pf{l}_{c}",
                             tag="pfpA" if c in (0, 3) else "pfpB")
                F = P * c
                mm(fc[0:64, :], vt[0:64, F:F + 64], G_prev[0:64, cs])
                mm(fc[64:128, :], vt[64:128, F + 64:F + 128], G_prev[64:128, cs])
                # per-chunk G pipeline: sech2 + s-fold, right behind the tanh
                hqc = scr.tile([P, BC], fp16, name=f"hq{l}_{c}", tag="hqc")
                mul(hqc, hl[:, cs], hl[:, cs])
                scc = scr.tile([P, BC], fp16, name=f"sc{l}_{c}", tag="scc")
                ts(scc, hqc, -1.0, 1.0, op0=ALU.mult, op1=ALU.add)
                stt(Gl[:, cs], fc, s[:, c:c + 1], scc,
                    op0=ALU.mult, op1=ALU.mult)
            return hl, Gl, tanhs

        h2, G2, T2 = big_batch(2, vt2, s2, h1, G1)

        vsq3, A_dA3, A_dB3 = prep_stat(3, vt3)
        s3 = prep_norms(3, vsq3)

        h3, G3, T3 = big_batch(3, vt3, s3, h2, G2)

        # ================= layer 4 batch (fully transposed) ===============
        z4 = pn.tile([P, 16], f32, name="z4", tag="pn")
        for b in range(2):
            # bias4 injected via a K=1 ones-row matmul (starts the group)
            mm(z4[:, 8 * b:8 * b + 8], onesr, sm("b4rep")[0:1, 8 * b:8 * b + 8],
               start=True, stop=False, skip_group_check=True)
            for k in range(4):
                mm(z4[:, 8 * b:8 * b + 8],
                   h3[:, BC * k + P * b:BC * k + P * b + P],
                   vt4s[:, 8 * k:8 * k + 8], start=False, stop=(k == 3),
                   skip_group_check=True)
        h4 = wgt.tile([P, 16], f32, name="h4", tag="h4")
        T4 = act(h4, z4, E.Tanh)
        nc.sync.dma_start(t["h4T_out"], h4)
        p4 = ptr.tile([P, 16], f32, name="p4", tag="ptr")
        for b in range(2):
            for k in range(4):
                mm(p4[:, 8 * b:8 * b + 8],
                   G3[:, BC * k + P * b:BC * k + P * b + P],
                   vd4s[:, 8 * k:8 * k + 8], start=(k == 0), stop=(k == 3))
        with tc.high_priority():
            # gt = p4 * (1 - h4^2) in two fused stt ops; s4 already in vd4s
            nh4 = scr.tile([P, 16], f32, name="nh4", tag="nh4")
            stt(nh4, h4, -1.0, h4, op0=ALU.mult, op1=ALU.mult)
            gt = wgt.tile([P, 16], f32, name="gt", tag="gt")
            stt(gt, nh4, 1.0, p4, op0=ALU.add, op1=ALU.mult)
            # fast log: ln(x) ~= LN2_A * float(bits(x)) + LN2_B
            gf = scr.tile([P, 16], f32, name="gf", tag="gf")
            cp(gf, gt.bitcast(u32))
            sld = wgt.tile([P, 16], f32, name="sld", tag="sld")
            ts(sld, gf, LN2_A, LN2_B, op0=ALU.mult, op1=ALU.add)
        nc.sync.dma_start(t["sldT_out"], sld)

        # ---- pin the ACT queue order: T1c0 can run before the l2 exps
        # (their weight-DMA sem lands later than T1c0's inputs); tanhs
        # otherwise never queue behind the next layer's prep exps ----
        chain = ([A_esm, T1[0], A_dA2, A_dB2, T1[1], T1[2], T1[3],
                  T2[0], A_dA3, A_dB3, T2[1], T2[2], T2[3]]
                 + T3 + [T4])
        for a, b in zip(chain, chain[1:]):
            dep(a, b)

    nc.compile()
    return nc


def _host_prep(x, W1, logg1, bias1, W2, logg2, bias2, W3, logg3, bias3,
               W4, logg4, bias4):
    """Pure layout prep (transpose / reshape / gather / masks), no arithmetic."""
    f = np.float32

    def cols(a):          # [512]-ish vector -> [128, 4] column-chunk layout
        return np.ascontiguousarray(np.reshape(a, (4, P)).T).astype(f)

    def fold(m):          # [512, 8] -> [128, (k x)] with k = row-chunk
        return m.reshape(4, P, 8).transpose(1, 0, 2).reshape(P, 32)

    def pack(WT):         # [512, 512] W.T -> [128, 1280] diag strip + windows
        wp = np.empty((P, 1280), f)
        for k in range(4):
            d = np.array(WT[P * k:P * k + P, P * k:P * k + P])
            d[64:128, 0:64] = 0.0          # structural mask: LL quadrant
            wp[:, P * k:P * k + P] = d
        for k in range(3):
            w = 384 - 128 * k
            wp[:, OFF2[k]:OFF2[k] + w] = WT[P * k:P * k + P, P * (k + 1):H]
        return wp

    smalls = np.zeros((P, SMALL_W), f)

    def put(name, arr):
        a, b = _SM[name]
        smalls[:arr.shape[0], a:b] = arr

    put("w1dg", cols(W1[np.arange(H), np.arange(H) // 64]))
    put("lg1", cols(logg1)); put("b1", cols(bias1))
    put("lg2", cols(logg2)); put("b2", cols(bias2))
    put("lg3", cols(logg3)); put("b3", cols(bias3))
    smalls[0, _SM["lg4r"][0]:_SM["lg4r"][0] + 8] = np.asarray(logg4).reshape(8)
    put("b4rep", np.concatenate([np.asarray(bias4).reshape(8)] * 2).reshape(1, 16))
    # structural masks (pre-applied on the host; -100 marks exp->0 positions)
    o = np.arange(H)[:, None] // 64
    i1 = np.arange(8)[None, :]
    md1 = (i1 == o); mo1 = (i1 < o)                        # [512, 8] natural
    W1n = np.asarray(W1).astype(f)
    put("w1mdN", fold(np.where(md1, W1n, f(-100.0))))
    put("w1moN", fold(np.where(mo1, W1n, f(0.0))))
    W4T = np.ascontiguousarray(np.asarray(W4).T).astype(f)  # [512, 8]
    ii = np.arange(H)[:, None] // 64
    o4 = np.arange(8)[None, :]
    md4 = (o4 == ii); mo4 = (o4 > ii)
    put("w4mdT", fold(np.where(md4, W4T, f(-100.0))))
    put("w4moT", fold(np.where(mo4, W4T, f(0.0))))

    wp2 = pack(np.ascontiguousarray(np.asarray(W2).T).astype(f))
    wp3 = pack(np.ascontiguousarray(np.asarray(W3).T).astype(f))
    xT = np.ascontiguousarray(np.asarray(x).T).astype(f)   # [8, 2048]
    return xT, wp2, wp3, smalls


def kernel(**inputs):
    global LAST_RESULTS
    from concourse.bass_utils import run_bass_kernel_spmd

    xT, wp2, wp3, smalls = _host_prep(**{k: np.asarray(v) for k, v in inputs.items()})

    if "nc" not in _CACHE:
        _CACHE["nc"] = _build()
    nc = _CACHE["nc"]

    wp2d = np.ascontiguousarray(wp2[:, 0:512]); wp2w = np.ascontiguousarray(wp2[:, 512:1280])
    wp3d = np.ascontiguousarray(wp3[:, 0:512]); wp3w = np.ascontiguousarray(wp3[:, 512:1280])
    in_maps = []
    for c in range(NCORE):
        # x.T row-replicated 16x so rows 32c+i == x.T[i] for the 32-aligned
        # L1 stationary slices
        xt4 = np.ascontiguousarray(np.tile(xT[:, BC * c:BC * (c + 1)], (16, 1)))
        in_maps.append({
            "xT4": xt4, "wp2d": wp2d, "wp2w": wp2w, "wp3d": wp3d,
            "wp3w": wp3w, "smalls": smalls,
        })
    res = run_bass_kernel_spmd(nc, in_maps, core_ids=list(range(NCORE)),
                               trace=TRACE)
    LAST_RESULTS = res

    B = BC * NCORE
    h = np.empty((B, 8), np.float32)
    sld = np.empty((B, 8), np.float32)
    for c, r in enumerate(res.results):
        h4 = r["h4T_out"]          # [128, 16]: h[128b+p, o] = h4[p, 8b+o]
        sl = r["sldT_out"]
        for b in range(2):
            h[BC * c + P * b: BC * c + P * (b + 1)] = h4[:, 8 * b:8 * b + 8]
            sld[BC * c + P * b: BC * c + P * (b + 1)] = sl[:, 8 * b:8 * b + 8]
    return h, sld


# revision 35
# speedup vs baseline: 1.0403x; 1.0403x over previous
"""BNAF forward (B=2048, D=8, H=512, 4 masked layers) on 8 TRN2 NeuronCores.

Strategy
--------
Pure data parallel: batch is split 256/core; the small weights are replicated.

Math: the BNAF log-det recursion collapses in exp space.  For each masked
linear layer, exp(logdet diag blocks) == the diag blocks of the normalized
weight w itself, and for tanh, exp(logdet) == 1 - h^2.  So the whole
log-sum-exp flow is a chain of *positive* block-diagonal matmuls with one
log() at the very end.  The per-output norm scale s = exp(logg)/||v|| is
folded into the G-flow moving operand (G_in = s * G), so the G-flow
stationary is just exp(W) block-diag.

v2 schedule notes (on top of the v1 design):
- the ACT queue order is pinned explicitly (esm exp -> l2 diag exps ->
  l1 tanhs -> l3 diag exps -> l2 tanhs -> l3 tanhs -> l4 tanh) with
  order-only dep edges, so layer-l tanhs are never stuck behind layer-l+1
  prep in the in-order ACT queue.
- vsq (norm squares) moved from ACT to DVE (one fp16 tensor_tensor per
  layer over the whole packed strip).
- the layer-1 stationary is built by exp-ing the natural [128,32] masked
  layout (part of the single batched esm exp) and PE-transposing it into
  [8,512]; this kills the slow 8-partition [8,512] EXP+ADD chain and the
  separate w1s DMA.
- bias4 is injected into the z4/PSUM accumulation with a K=1 ones-row
  matmul; the final tail is 2 fused stt ops + fast-log.
- smalls DMA split so the exp block lands first; weight cast-DMAs
  reordered (vt2 diag first), xT moved to the scalar queue.
- ACT only ever uses {Exp, Tanh}: single table load at start.
"""

import numpy as np

TRACE = False          # set by test.py for profiling runs
LAST_RESULTS = None    # BassKernelResults stash for test.py

_CACHE = {}

P = 128
BC = 256          # batch per core
H = 512
NCORE = 8
MAGIC = 0x5f3759df
OFF2 = (512, 896, 1152)   # packed col offsets of the strictly-lower windows
LN2_A = 8.262958294867817e-08     # ln2 * 2^-23
LN2_B = -90.77247532458875        # -126.9570 * ln2 - 4*ln2 (G-flow 2^4 scale)

# smalls layout: first the exp block (exp'd in one ACT op), then the rest,
# then the l2/l3 diag strips (exp-able quadrants + raw UR quadrants) so the
# whole latency-critical small-weight path rides ONE fast HWDGE DMA.
# wNmd* entries hold where(mask_d, W, -100): exp gives exp(W)*mask_d exactly,
# so the masked-linear weights need no on-device mask multiplies.
_SM = {}
_off = 0
for _name, _w in [("w1dg", 4), ("lg1", 4), ("lg2", 4), ("lg3", 4),
                  ("lg4r", 8), ("w1mdN", 32), ("w4mdT", 32),      # exp block
                  ("b1", 4), ("b2", 4), ("b3", 4), ("b4rep", 16),
                  ("w1moN", 32), ("w4moT", 32)]:
    _SM[_name] = (_off, _off + _w)
    _off += _w
SMALL_W = _off
EXPW = _SM["w4mdT"][1]        # width of the exp block (88)


def _vsl(vt, k, c):
    """Packed-layout slice of the (in-chunk k, out-chunk c) 128x128 block."""
    if k == c:
        return vt[:, 128 * k:128 * k + 128]
    o = OFF2[k] + 128 * (c - k - 1)
    return vt[:, o:o + 128]


def _build():
    import concourse.bacc as bacc
    import concourse.mybir as mybir
    import concourse.tile as tile
    from concourse.tile_rust import add_dep_helper
    from concourse.masks import make_identity
    from contextlib import ExitStack

    f32 = mybir.dt.float32
    u32 = mybir.dt.uint32
    fp16 = mybir.dt.float16
    E = mybir.ActivationFunctionType
    ALU = mybir.AluOpType

    nc = bacc.Bacc("TRN2", target_bir_lowering=False, debug=False,
                   enable_asserts=False, num_devices=NCORE)

    t = {}
    t["xT4"] = nc.dram_tensor("xT4", (P, BC), f32, kind="ExternalInput").ap()
    t["wp2d"] = nc.dram_tensor("wp2d", (P, H), f32, kind="ExternalInput").ap()
    t["wp2w"] = nc.dram_tensor("wp2w", (P, 768), f32, kind="ExternalInput").ap()
    t["wp3d"] = nc.dram_tensor("wp3d", (P, H), f32, kind="ExternalInput").ap()
    t["wp3w"] = nc.dram_tensor("wp3w", (P, 768), f32, kind="ExternalInput").ap()
    t["smalls"] = nc.dram_tensor("smalls", (P, SMALL_W), f32, kind="ExternalInput").ap()
    t["h4T_out"] = nc.dram_tensor("h4T_out", (P, 16), f32, kind="ExternalOutput").ap()
    t["sldT_out"] = nc.dram_tensor("sldT_out", (P, 16), f32, kind="ExternalOutput").ap()

    def mm(out, lhsT, rhs, **kw):
        return nc.tensor.matmul(out, lhsT, rhs, **kw)

    def dep(a, b):
        """Pin engine-queue order: instruction a runs before b."""
        add_dep_helper(b.ins, a.ins, False, "act-order")

    with tile.TileContext(nc) as tc, ExitStack() as ctx:
        wgt = ctx.enter_context(tc.tile_pool(name="wgt", bufs=1))
        scr = ctx.enter_context(tc.tile_pool(name="scr", bufs=3))
        pz = ctx.enter_context(tc.tile_pool(name="pz", bufs=1, space="PSUM"))
        pf = ctx.enter_context(tc.tile_pool(name="pf", bufs=1, space="PSUM"))
        pn = ctx.enter_context(tc.tile_pool(name="pn", bufs=1, space="PSUM"))
        ptr = ctx.enter_context(tc.tile_pool(name="ptr", bufs=1, space="PSUM"))

        act = nc.scalar.activation
        cp = nc.vector.tensor_copy
        ts = nc.vector.tensor_scalar
        stt = nc.vector.scalar_tensor_tensor
        mul = nc.vector.tensor_mul
        tt = nc.vector.tensor_tensor

        # ---- input DMAs ----
        # smalls on the fast first HWDGE DMA (sem ~9.2us); x (row-replicated
        # 4x for the 32-aligned L1 stationary slices) on the scalar queue
        # (sem ~9.6us); the packed weight strips as gpsimd fp32->fp16
        # cast-DMAs (sems ~11.2us + ~0.6/queue-slot).
        smalls = wgt.tile([P, SMALL_W], f32, name="smalls_t", tag="smalls_t")
        nc.sync.dma_start(smalls, t["smalls"])
        xT4 = wgt.tile([P, BC], f32, name="xT4", tag="xT4")
        nc.scalar.dma_start(xT4, t["xT4"])
        # gpsimd queue: tiny memsets + identity first, then the weight DMAs
        wz = wgt.tile([P, BC], fp16, name="wz", tag="wz")
        nc.gpsimd.memset(wz, 0.0)
        ident = wgt.tile([P, P], f32, name="ident", tag="ident")
        make_identity(nc, ident)
        vt2 = wgt.tile([P, 1280], fp16, name="vt2", tag="vt2")
        vt3 = wgt.tile([P, 1280], fp16, name="vt3", tag="vt3")
        nc.gpsimd.dma_start(vt2[:, 0:H], t["wp2d"])
        nc.gpsimd.dma_start(vt2[:, H:1280], t["wp2w"])
        nc.gpsimd.dma_start(vt3[:, 0:H], t["wp3d"])
        nc.gpsimd.dma_start(vt3[:, H:1280], t["wp3w"])

        def sm(name):
            a, b = _SM[name]
            return smalls[:, a:b]

        # ---- tiny constants on DVE (keep Q7 free) ----
        magict = wgt.tile([P, 8], u32, name="magict", tag="magict")
        nc.vector.memset(magict, MAGIC)
        ones4f = wgt.tile([P, 4], f32, name="ones4f", tag="ones4f")
        nc.vector.memset(ones4f, 1.0)
        ones4 = wgt.tile([P, 4], fp16, name="ones4", tag="ones4")
        cp(ones4, ones4f)
        onesr = wgt.tile([1, P], f32, name="onesr", tag="onesr")
        nc.vector.memset(onesr, 1.0)

        # short PE warm-up burst (HAM un-throttle) while DMAs drain
        pw = pn.tile([2, BC - 2], f32, name="pw", tag="pn")
        for _ in range(12):
            mm(pw, wz[:, 0:2], wz[:, 2:BC], skip_group_check=True)

        # one batched exp over the whole exp block
        esm = wgt.tile([P, EXPW], f32, name="esm", tag="esm")
        A_esm = act(esm, smalls[:, 0:EXPW], E.Exp)

        def esl(name):
            a, b = _SM[name]
            return esm[:, a:b]

        e1d = esl("w1dg")
        eg = {1: esl("lg1"), 2: esl("lg2"), 3: esl("lg3")}
        eg4row = esm[0:1, _SM["lg4r"][0]:_SM["lg4r"][0] + 8]
        e1mdN = esl("w1mdN")
        e4mdT = esl("w4mdT")

        # s = eg * rsqrt(norm2): DVE-only Newton rsqrt (reads n2 psum directly)
        def make_scale(n2_ap, eg_ap, shape, nm):
            pr = shape[0]
            shf = scr.tile(list(shape), u32, name=f"shf_{nm}", tag="sc_shf")
            ts(shf, n2_ap.bitcast(u32), 1, None, op0=ALU.arith_shift_right)
            y0 = scr.tile(list(shape), u32, name=f"y0_{nm}", tag="sc_y0")
            stt(y0, magict[:pr, :shape[1]], 0, shf, op0=ALU.bypass, op1=ALU.subtract)
            y = y0.bitcast(f32)
            t1 = scr.tile(list(shape), f32, name=f"t1_{nm}", tag="sc_t1")
            t2 = scr.tile(list(shape), f32, name=f"t2_{nm}", tag="sc_t2")
            for it in range(1):         # one Newton step: y *= 1.5 - 0.5*n2*y*y
                mul(t1, y, y)
                mul(t2, t1, n2_ap)
                ts(t1, t2, -0.5, 1.5, op0=ALU.mult, op1=ALU.add)
                yn = scr.tile(list(shape), f32, name=f"yn{it}_{nm}", tag=f"sc_yn{it}")
                mul(yn, y, t1)
                y = yn
            s = wgt.tile(list(shape), f32, name=f"s_{nm}", tag=f"s_{nm}")
            mul(s, eg_ap, y)
            return s

        # ================= layer 1 prep ===================================
        # v1n in a 32-col-padded natural layout (chunk c at cols 32c:32c+8):
        # one strided add, ONE PE transpose, one copy.  The L1 stationary is
        # then v1TP[32c:32c+8, :] (32-aligned partition slices, matching the
        # row-replicated xT4 moving operand), consumed as f32r.
        with tc.high_priority():
            v1nP = wgt.tile([P, P], f32, name="v1nP", tag="v1nP")
            v1nv = v1nP.rearrange("p (b c) -> p b c", c=32)[:, :, 0:8]
            tt(v1nv, e1mdN.rearrange("p (b c) -> p b c", c=8),
               sm("w1moN").rearrange("p (b c) -> p b c", c=8), op=ALU.add)
            n1 = wgt.tile([P, 4], f32, name="n1", tag="n1")
            for c in range(4):
                sq1 = scr.tile([P, 8], f32, name=f"sq1_{c}", tag="sq1")
                stt(sq1, v1nP[:, 32 * c:32 * c + 8], 0, v1nP[:, 32 * c:32 * c + 8],
                    op0=ALU.bypass, op1=ALU.mult, accum_out=n1[:, c:c + 1])
            s1 = make_scale(n1, eg[1], (P, 4), "l1")
            e1s = wgt.tile([P, 4], f32, name="e1s", tag="e1s")
            E1S = stt(e1s, e1d, 16.0, s1, op0=ALU.mult, op1=ALU.mult)
            # chunk 3 would land at base partition 96 (invalid: bases must be
            # 0/32/64), so it gets its own base-0 transpose of the last slice;
            # both transposes share one PSUM tile -> a single copy out
            ptp = ptr.tile([P, 2 * P], f32, name="ptp", tag="ptr")
            nc.tensor.transpose(ptp[:, 0:P], v1nP, ident)
            nc.tensor.transpose(ptp[0:32, P:2 * P], v1nP[:, 96:128], ident)
            v1TA = wgt.tile([P, 2 * P], fp16, name="v1TA", tag="v1TA")
            cp(v1TA, ptp)
            xT4h = wgt.tile([P, BC], fp16, name="xT4h", tag="xT4h")
            cp(xT4h, xT4)

        # ======= layer 4 prep (cheap; pinned AFTER the layer-1 DVE chain so
        # it never wedges the s1 path, but early enough to be ready) =======
        if True:
            vt4 = wgt.tile([P, 32], fp16, name="vt4", tag="vt4")
            VT4 = tt(vt4, e4mdT, sm("w4moT"), op=ALU.add)
            dep(E1S, VT4)
            vsq4 = scr.tile([P, 32], fp16, name="vsq4", tag="vsq4")
            mul(vsq4, vt4, vt4)
            n4 = pn.tile([1, 8], f32, name="n4", tag="pn")
            for k in range(4):
                mm(n4, ones4[:, 0:1], vsq4[:, 8 * k:8 * k + 8],
                   start=(k == 0), stop=(k == 3))
            s4r = make_scale(n4, eg4row, (1, 8), "l4")
            # broadcast s4 to all partitions with a K=1 ones-row matmul
            s4x = pn.tile([P, 32], f32, name="s4x", tag="pn")
            for k in range(4):
                mm(s4x[:, 8 * k:8 * k + 8], onesr, s4r)
            # fold s4 into both layer-4 stationaries (scale along out-feats)
            vt4s = wgt.tile([P, 32], fp16, name="vt4s", tag="vt4s")
            mul(vt4s, vt4, s4x)
            vd4s = wgt.tile([P, 32], fp16, name="vd4s", tag="vd4s")
            mul(vd4s, e4mdT, s4x)

        # ================= layer 1 batch ==================================
        h1 = wgt.tile([P, 4 * BC], fp16, name="h1", tag="h1")
        G1 = wgt.tile([P, 4 * BC], fp16, name="G1", tag="G1")
        T1 = []
        for c in range(4):
            cs = slice(BC * c, BC * c + BC)
            zc = pz.tile([P, BC], f32, name=f"pz1_{c}", tag=f"pzc{c}")
            if c < 3:
                mm(zc, v1TA[32 * c:32 * c + 8, 0:P], xT4h[32 * c:32 * c + 8, :])
            else:
                mm(zc, v1TA[0:8, P:2 * P], xT4h[0:8, :])
            T1.append(act(h1[:, cs], zc, E.Tanh,
                          bias=sm("b1")[:, c:c + 1], scale=s1[:, c:c + 1]))
            # G1 = (1 - h^2) * e1s: 2 fused DVE ops per chunk
            nh2 = scr.tile([P, BC], fp16, name=f"nh1_{c}", tag="hqc")
            stt(nh2, h1[:, cs], -1.0, h1[:, cs], op0=ALU.mult, op1=ALU.mult)
            ts(G1[:, cs], nh2, 1.0, e1s[:, c:c + 1], op0=ALU.add, op1=ALU.mult)

        # ================= layer 2/3 prep ================================
        def prep_stat(l, vt):
            # in-place exp of the 8 diag 64x64 blocks: two strided ACT ops
            dA = vt[0:64, 0:H].rearrange("p (b c) -> p b c", c=128)[:, :, 0:64]
            a1 = act(dA, dA, E.Exp)
            dB = vt[64:128, 0:H].rearrange("p (b c) -> p b c", c=128)[:, :, 64:128]
            a2 = act(dB, dB, E.Exp)
            # vsq on DVE, split so the window part (gated only by its DMA)
            # can run before the diag part (gated by the exps); both pinned
            # behind the layer-1 DVE chain so they can't wedge it
            vsq = scr.tile([P, 1280], fp16, name=f"vsq{l}", tag="vsq")
            vw = mul(vsq[:, H:1280], vt[:, H:1280], vt[:, H:1280])
            vd = mul(vsq[:, 0:H], vt[:, 0:H], vt[:, 0:H])
            dep(E1S, vw)
            dep(E1S, vd)
            return vsq, a1, a2

        def prep_norms(l, vsq):
            # norm2 directly in partition layout: vsq-stationary x ones-col
            ncol = pn.tile([P, 4], f32, name=f"ncol{l}", tag="pn")
            with tc.high_priority():
                for c in range(4):
                    for k in range(c + 1):
                        mm(ncol[:, c:c + 1], _vsl(vsq, k, c), ones4[:, 0:1],
                           start=(k == 0), stop=(k == c))
                return make_scale(ncol, eg[l], (P, 4), f"l{l}")

        vsq2, A_dA2, A_dB2 = prep_stat(2, vt2)
        s2 = prep_norms(2, vsq2)

        # ================= layer 2/3 batch ================================
        def big_batch(l, vt, s, h_prev, G_prev):
            hl = wgt.tile([P, 4 * BC], fp16, name=f"h{l}", tag=f"h{l}")
            Gl = wgt.tile([P, 4 * BC], fp16, name=f"G{l}", tag=f"G{l}")
            tanhs = []
            for c in range(4):
                cs = slice(BC * c, BC * c + BC)
                zc = pz.tile([P, BC], f32, name=f"pz{l}_{c}", tag=f"pzc{c}")
                for k in range(c + 1):
                    mm(zc, _vsl(vt, k, c), h_prev[:, BC * k:BC * k + BC],
                       start=(k == 0), stop=(k == c))
                tanhs.append(act(hl[:, cs], zc, E.Tanh,
                                 bias=sm(f"b{l}")[:, c:c + 1], scale=s[:, c:c + 1]))
                # pair (c0,c3) and (c1,c2): c3's buffer is freed by c0's
                # early Gin read, so the late G-tail never stalls on PSUM
                fc = pf.tile([P, BC], f32, name=f"pf{l}_{c}",
                             tag="pfpA" if c in (0, 3) else "pfpB")
                F = P * c
                mm(fc[0:64, :], vt[0:64, F:F + 64], G_prev[0:64, cs])
                mm(fc[64:128, :], vt[64:128, F + 64:F + 128], G_prev[64:128, cs])
                # per-chunk G pipeline: sech2 + s-fold, right behind the tanh
                hqc = scr.tile([P, BC], fp16, name=f"hq{l}_{c}", tag="hqc")
                mul(hqc, hl[:, cs], hl[:, cs])
                scc = scr.tile([P, BC], fp16, name=f"sc{l}_{c}", tag="scc")
                ts(scc, hqc, -1.0, 1.0, op0=ALU.mult, op1=ALU.add)
                stt(Gl[:, cs], fc, s[:, c:c + 1], scc,
                    op0=ALU.mult, op1=ALU.mult)
            return hl, Gl, tanhs

        h2, G2, T2 = big_batch(2, vt2, s2, h1, G1)

        vsq3, A_dA3, A_dB3 = prep_stat(3, vt3)
        s3 = prep_norms(3, vsq3)

        h3, G3, T3 = big_batch(3, vt3, s3, h2, G2)

        # ================= layer 4 batch (fully transposed) ===============
        z4 = pn.tile([P, 16], f32, name="z4", tag="pn")
        for b in range(2):
            # bias4 injected via a K=1 ones-row matmul (starts the group)
            mm(z4[:, 8 * b:8 * b + 8], onesr, sm("b4rep")[0:1, 8 * b:8 * b + 8],
               start=True, stop=False, skip_group_check=True)
            for k in range(4):
                mm(z4[:, 8 * b:8 * b + 8],
                   h3[:, BC * k + P * b:BC * k + P * b + P],
                   vt4s[:, 8 * k:8 * k + 8], start=False, stop=(k == 3),
                   skip_group_check=True)
        h4 = wgt.tile([P, 16], f32, name="h4", tag="h4")
        T4 = act(h4, z4, E.Tanh)
        nc.sync.dma_start(t["h4T_out"], h4)
        p4 = ptr.tile([P, 16], f32, name="p4", tag="ptr")
        for b in range(2):
            for k in range(4):
                mm(p4[:, 8 * b:8 * b + 8],
                   G3[:, BC * k + P * b:BC * k + P * b + P],
                   vd4s[:, 8 * k:8 * k + 8], start=(k == 0), stop=(k == 3))
        with tc.high_priority():
            # gt = p4 * (1 - h4^2) in two fused stt ops; s4 already in vd4s
            nh4 = scr.tile([P, 16], f32, name="nh4", tag="nh4")
            stt(nh4, h4, -1.0, h4, op0=ALU.mult, op1=ALU.mult)
            gt = wgt.tile([P, 16], f32, name="gt", tag="gt")
            stt(gt, nh4, 1.0, p4, op0=ALU.add, op1=ALU.mult)
            # fast log: ln(x) ~= LN2_A * float(bits(x)) + LN2_B
            gf = scr.tile([P, 16], f32, name="gf", tag="gf")
            cp(gf, gt.bitcast(u32))
            sld = wgt.tile([P, 16], f32, name="sld", tag="sld")
            ts(sld, gf, LN2_A, LN2_B, op0=ALU.mult, op1=ALU.add)
        nc.sync.dma_start(t["sldT_out"], sld)

        # ---- pin the ACT queue order: interleave the l2/l3 prep exps with
        # the tanhs so neither starves the other ----
        chain = ([A_esm, A_dA2, T1[0], A_dB2, T1[1], T1[2], T1[3],
                  T2[0], A_dA3, A_dB3, T2[1], T2[2], T2[3]]
                 + T3 + [T4])
        for a, b in zip(chain, chain[1:]):
            dep(a, b)

    nc.compile()
    return nc


def _host_prep(x, W1, logg1, bias1, W2, logg2, bias2, W3, logg3, bias3,
               W4, logg4, bias4):
    """Pure layout prep (transpose / reshape / gather / masks), no arithmetic."""
    f = np.float32

    def cols(a):          # [512]-ish vector -> [128, 4] column-chunk layout
        return np.ascontiguousarray(np.reshape(a, (4, P)).T).astype(f)

    def fold(m):          # [512, 8] -> [128, (k x)] with k = row-chunk
        return m.reshape(4, P, 8).transpose(1, 0, 2).reshape(P, 32)

    def pack(WT):         # [512, 512] W.T -> [128, 1280] diag strip + windows
        wp = np.empty((P, 1280), f)
        for k in range(4):
            d = np.array(WT[P * k:P * k + P, P * k:P * k + P])
            d[64:128, 0:64] = 0.0          # structural mask: LL quadrant
            wp[:, P * k:P * k + P] = d
        for k in range(3):
            w = 384 - 128 * k
            wp[:, OFF2[k]:OFF2[k] + w] = WT[P * k:P * k + P, P * (k + 1):H]
        return wp

    smalls = np.zeros((P, SMALL_W), f)

    def put(name, arr):
        a, b = _SM[name]
        smalls[:arr.shape[0], a:b] = arr

    put("w1dg", cols(W1[np.arange(H), np.arange(H) // 64]))
    put("lg1", cols(logg1)); put("b1", cols(bias1))
    put("lg2", cols(logg2)); put("b2", cols(bias2))
    put("lg3", cols(logg3)); put("b3", cols(bias3))
    smalls[0, _SM["lg4r"][0]:_SM["lg4r"][0] + 8] = np.asarray(logg4).reshape(8)
    put("b4rep", np.concatenate([np.asarray(bias4).reshape(8)] * 2).reshape(1, 16))
    # structural masks (pre-applied on the host; -100 marks exp->0 positions)
    o = np.arange(H)[:, None] // 64
    i1 = np.arange(8)[None, :]
    md1 = (i1 == o); mo1 = (i1 < o)                        # [512, 8] natural
    W1n = np.asarray(W1).astype(f)
    put("w1mdN", fold(np.where(md1, W1n, f(-100.0))))
    put("w1moN", fold(np.where(mo1, W1n, f(0.0))))
    W4T = np.ascontiguousarray(np.asarray(W4).T).astype(f)  # [512, 8]
    ii = np.arange(H)[:, None] // 64
    o4 = np.arange(8)[None, :]
    md4 = (o4 == ii); mo4 = (o4 > ii)
    put("w4mdT", fold(np.where(md4, W4T, f(-100.0))))
    put("w4moT", fold(np.where(mo4, W4T, f(0.0))))

    wp2 = pack(np.ascontiguousarray(np.asarray(W2).T).astype(f))
    wp3 = pack(np.ascontiguousarray(np.asarray(W3).T).astype(f))
    xT = np.ascontiguousarray(np.asarray(x).T).astype(f)   # [8, 2048]
    return xT, wp2, wp3, smalls


def kernel(**inputs):
    global LAST_RESULTS
    from concourse.bass_utils import run_bass_kernel_spmd

    xT, wp2, wp3, smalls = _host_prep(**{k: np.asarray(v) for k, v in inputs.items()})

    if "nc" not in _CACHE:
        _CACHE["nc"] = _build()
    nc = _CACHE["nc"]

    wp2d = np.ascontiguousarray(wp2[:, 0:512]); wp2w = np.ascontiguousarray(wp2[:, 512:1280])
    wp3d = np.ascontiguousarray(wp3[:, 0:512]); wp3w = np.ascontiguousarray(wp3[:, 512:1280])
    in_maps = []
    for c in range(NCORE):
        # x.T row-replicated 16x so rows 32c+i == x.T[i] for the 32-aligned
        # L1 stationary slices
        xt4 = np.ascontiguousarray(np.tile(xT[:, BC * c:BC * (c + 1)], (16, 1)))
        in_maps.append({
            "xT4": xt4, "wp2d": wp2d, "wp2w": wp2w, "wp3d": wp3d,
            "wp3w": wp3w, "smalls": smalls,
        })
    res = run_bass_kernel_spmd(nc, in_maps, core_ids=list(range(NCORE)),
                               trace=TRACE)
    LAST_RESULTS = res

    B = BC * NCORE
    h = np.empty((B, 8), np.float32)
    sld = np.empty((B, 8), np.float32)
    for c, r in enumerate(res.results):
        h4 = r["h4T_out"]          # [128, 16]: h[128b+p, o] = h4[p, 8b+o]
        sl = r["sldT_out"]
        for b in range(2):
            h[BC * c + P * b: BC * c + P * (b + 1)] = h4[:, 8 * b:8 * b + 8]
            sld[BC * c + P * b: BC * c + P * (b + 1)] = sl[:, 8 * b:8 * b + 8]
    return h, sld


# revision 45
# speedup vs baseline: 1.0678x; 1.0264x over previous
"""BNAF forward (B=2048, D=8, H=512, 4 masked layers) on 8 TRN2 NeuronCores.

Strategy
--------
Pure data parallel: batch is split 256/core; the small weights are replicated.

Math: the BNAF log-det recursion collapses in exp space.  For each masked
linear layer, exp(logdet diag blocks) == the diag blocks of the normalized
weight w itself, and for tanh, exp(logdet) == 1 - h^2.  So the whole
log-sum-exp flow is a chain of *positive* block-diagonal matmuls with one
log() at the very end.  The per-output norm scale s = exp(logg)/||v|| is
folded into the G-flow moving operand (G_in = s * G), so the G-flow
stationary is just exp(W) block-diag.

v2 schedule notes (on top of the v1 design):
- the ACT queue order is pinned explicitly (esm exp -> l2 diag exps ->
  l1 tanhs -> l3 diag exps -> l2 tanhs -> l3 tanhs -> l4 tanh) with
  order-only dep edges, so layer-l tanhs are never stuck behind layer-l+1
  prep in the in-order ACT queue.
- vsq (norm squares) moved from ACT to DVE (one fp16 tensor_tensor per
  layer over the whole packed strip).
- the layer-1 stationary is built by exp-ing the natural [128,32] masked
  layout (part of the single batched esm exp) and PE-transposing it into
  [8,512]; this kills the slow 8-partition [8,512] EXP+ADD chain and the
  separate w1s DMA.
- bias4 is injected into the z4/PSUM accumulation with a K=1 ones-row
  matmul; the final tail is 2 fused stt ops + fast-log.
- smalls DMA split so the exp block lands first; weight cast-DMAs
  reordered (vt2 diag first), xT moved to the scalar queue.
- ACT only ever uses {Exp, Tanh}: single table load at start.
"""

import numpy as np

TRACE = False          # set by test.py for profiling runs
LAST_RESULTS = None    # BassKernelResults stash for test.py

_CACHE = {}

P = 128
BC = 256          # batch per core
H = 512
NCORE = 8
MAGIC = 0x5f3759df
OFF2 = (512, 896, 1152)   # packed col offsets of the strictly-lower windows
LN2_A = 8.262958294867817e-08     # ln2 * 2^-23
LN2_B = -90.77247532458875        # -126.9570 * ln2 - 4*ln2 (G-flow 2^4 scale)

# smalls layout: first the exp block (exp'd in one ACT op), then the rest,
# then the l2/l3 diag strips (exp-able quadrants + raw UR quadrants) so the
# whole latency-critical small-weight path rides ONE fast HWDGE DMA.
# wNmd* entries hold where(mask_d, W, -100): exp gives exp(W)*mask_d exactly,
# so the masked-linear weights need no on-device mask multiplies.
_SM = {}
_off = 0
for _name, _w in [("w1dg", 4), ("lg1", 4), ("lg2", 4), ("lg3", 4),
                  ("lg4r", 8), ("w1mdN", 32), ("w4mdT", 32),      # exp block
                  ("b1", 4), ("b2", 4), ("b3", 4), ("b4rep", 16),
                  ("w1moN", 32), ("w4moT", 32)]:
    _SM[_name] = (_off, _off + _w)
    _off += _w
SMALL_W = _off
EXPW = _SM["w4mdT"][1]        # width of the exp block (88)


def _vsl(vt, k, c):
    """Packed-layout slice of the (in-chunk k, out-chunk c) 128x128 block."""
    if k == c:
        return vt[:, 128 * k:128 * k + 128]
    o = OFF2[k] + 128 * (c - k - 1)
    return vt[:, o:o + 128]


def _build():
    import concourse.bacc as bacc
    import concourse.mybir as mybir
    import concourse.tile as tile
    from concourse.tile_rust import add_dep_helper
    from concourse.masks import make_identity
    from contextlib import ExitStack

    f32 = mybir.dt.float32
    u32 = mybir.dt.uint32
    fp16 = mybir.dt.float16
    E = mybir.ActivationFunctionType
    ALU = mybir.AluOpType

    nc = bacc.Bacc("TRN2", target_bir_lowering=False, debug=False,
                   enable_asserts=False, num_devices=NCORE)

    t = {}
    t["xT4"] = nc.dram_tensor("xT4", (P, BC), f32, kind="ExternalInput").ap()
    t["wp2d"] = nc.dram_tensor("wp2d", (P, H), f32, kind="ExternalInput").ap()
    t["wp2w"] = nc.dram_tensor("wp2w", (P, 768), f32, kind="ExternalInput").ap()
    t["wp3d"] = nc.dram_tensor("wp3d", (P, H), f32, kind="ExternalInput").ap()
    t["wp3w"] = nc.dram_tensor("wp3w", (P, 768), f32, kind="ExternalInput").ap()
    t["smalls"] = nc.dram_tensor("smalls", (P, SMALL_W), f32, kind="ExternalInput").ap()
    t["h4T_out"] = nc.dram_tensor("h4T_out", (P, 16), f32, kind="ExternalOutput").ap()
    t["sldT_out"] = nc.dram_tensor("sldT_out", (P, 16), f32, kind="ExternalOutput").ap()

    def mm(out, lhsT, rhs, **kw):
        return nc.tensor.matmul(out, lhsT, rhs, **kw)

    def dep(a, b):
        """Pin engine-queue order: instruction a runs before b."""
        add_dep_helper(b.ins, a.ins, False, "act-order")

    with tile.TileContext(nc) as tc, ExitStack() as ctx:
        wgt = ctx.enter_context(tc.tile_pool(name="wgt", bufs=1))
        scr = ctx.enter_context(tc.tile_pool(name="scr", bufs=3))
        pz = ctx.enter_context(tc.tile_pool(name="pz", bufs=1, space="PSUM"))
        pf = ctx.enter_context(tc.tile_pool(name="pf", bufs=1, space="PSUM"))
        pn = ctx.enter_context(tc.tile_pool(name="pn", bufs=1, space="PSUM"))
        ptr = ctx.enter_context(tc.tile_pool(name="ptr", bufs=1, space="PSUM"))

        act = nc.scalar.activation
        cp = nc.vector.tensor_copy
        ts = nc.vector.tensor_scalar
        stt = nc.vector.scalar_tensor_tensor
        mul = nc.vector.tensor_mul
        tt = nc.vector.tensor_tensor

        # ---- input DMAs ----
        # smalls on the fast first HWDGE DMA (sem ~9.2us); x (row-replicated
        # 4x for the 32-aligned L1 stationary slices) on the scalar queue
        # (sem ~9.6us); the packed weight strips as gpsimd fp32->fp16
        # cast-DMAs (sems ~11.2us + ~0.6/queue-slot).
        smalls = wgt.tile([P, SMALL_W], f32, name="smalls_t", tag="smalls_t")
        nc.sync.dma_start(smalls, t["smalls"])
        xT4 = wgt.tile([P, BC], f32, name="xT4", tag="xT4")
        nc.scalar.dma_start(xT4, t["xT4"])
        # gpsimd queue: tiny memsets + identity first, then the weight DMAs
        wz = wgt.tile([P, BC], fp16, name="wz", tag="wz")
        nc.gpsimd.memset(wz, 0.0)
        ident = wgt.tile([P, P], f32, name="ident", tag="ident")
        make_identity(nc, ident)
        vt2 = wgt.tile([P, 1280], fp16, name="vt2", tag="vt2")
        vt3 = wgt.tile([P, 1280], fp16, name="vt3", tag="vt3")
        nc.gpsimd.dma_start(vt2[:, 0:H], t["wp2d"])
        nc.gpsimd.dma_start(vt2[:, H:1280], t["wp2w"])
        nc.gpsimd.dma_start(vt3[:, 0:H], t["wp3d"])
        nc.gpsimd.dma_start(vt3[:, H:1280], t["wp3w"])

        def sm(name):
            a, b = _SM[name]
            return smalls[:, a:b]

        # ---- tiny constants on DVE (keep Q7 free) ----
        magict = wgt.tile([P, 8], u32, name="magict", tag="magict")
        nc.vector.memset(magict, MAGIC)
        ones4f = wgt.tile([P, 4], f32, name="ones4f", tag="ones4f")
        nc.vector.memset(ones4f, 1.0)
        ones4 = wgt.tile([P, 4], fp16, name="ones4", tag="ones4")
        cp(ones4, ones4f)
        onesr = wgt.tile([1, P], f32, name="onesr", tag="onesr")
        nc.vector.memset(onesr, 1.0)

        # short PE warm-up burst (HAM un-throttle) while DMAs drain
        pw = pn.tile([2, BC - 2], f32, name="pw", tag="pn")
        for _ in range(12):
            mm(pw, wz[:, 0:2], wz[:, 2:BC], skip_group_check=True)

        # one batched exp over the whole exp block
        esm = wgt.tile([P, EXPW], f32, name="esm", tag="esm")
        A_esm = act(esm, smalls[:, 0:EXPW], E.Exp)

        def esl(name):
            a, b = _SM[name]
            return esm[:, a:b]

        e1d = esl("w1dg")
        eg = {1: esl("lg1"), 2: esl("lg2"), 3: esl("lg3")}
        eg4row = esm[0:1, _SM["lg4r"][0]:_SM["lg4r"][0] + 8]
        e1mdN = esl("w1mdN")
        e4mdT = esl("w4mdT")

        # s = eg * rsqrt(norm2): DVE-only Newton rsqrt (reads n2 psum directly)
        def make_scale(n2_ap, eg_ap, shape, nm):
            pr = shape[0]
            shf = scr.tile(list(shape), u32, name=f"shf_{nm}", tag=f"sc_shf_{nm}")
            ts(shf, n2_ap.bitcast(u32), 1, None, op0=ALU.arith_shift_right)
            y0 = scr.tile(list(shape), u32, name=f"y0_{nm}", tag=f"sc_y0_{nm}")
            stt(y0, magict[:pr, :shape[1]], 0, shf, op0=ALU.bypass, op1=ALU.subtract)
            y = y0.bitcast(f32)
            t1 = scr.tile(list(shape), f32, name=f"t1_{nm}", tag=f"sc_t1_{nm}")
            t2 = scr.tile(list(shape), f32, name=f"t2_{nm}", tag=f"sc_t2_{nm}")
            for it in range(1):         # one Newton step: y *= 1.5 - 0.5*n2*y*y
                mul(t1, y, y)
                mul(t2, t1, n2_ap)
                ts(t1, t2, -0.5, 1.5, op0=ALU.mult, op1=ALU.add)
                yn = scr.tile(list(shape), f32, name=f"yn{it}_{nm}", tag=f"sc_yn{it}_{nm}")
                mul(yn, y, t1)
                y = yn
            s = wgt.tile(list(shape), f32, name=f"s_{nm}", tag=f"s_{nm}")
            mul(s, eg_ap, y)
            return s

        # ================= layer 1 prep ===================================
        # v1n in a 32-col-padded natural layout (chunk c at cols 32c:32c+8):
        # one strided add, ONE PE transpose, one copy.  The L1 stationary is
        # then v1TP[32c:32c+8, :] (32-aligned partition slices, matching the
        # row-replicated xT4 moving operand), consumed as f32r.
        with tc.high_priority():
            v1nP = wgt.tile([P, P], f32, name="v1nP", tag="v1nP")
            v1nv = v1nP.rearrange("p (b c) -> p b c", c=32)[:, :, 0:8]
            tt(v1nv, e1mdN.rearrange("p (b c) -> p b c", c=8),
               sm("w1moN").rearrange("p (b c) -> p b c", c=8), op=ALU.add)
            n1 = wgt.tile([P, 4], f32, name="n1", tag="n1")
            for c in range(4):
                sq1 = scr.tile([P, 8], f32, name=f"sq1_{c}", tag=f"sq1_{c}")
                stt(sq1, v1nP[:, 32 * c:32 * c + 8], 0, v1nP[:, 32 * c:32 * c + 8],
                    op0=ALU.bypass, op1=ALU.mult, accum_out=n1[:, c:c + 1])
            s1 = make_scale(n1, eg[1], (P, 4), "l1")
            e1s = wgt.tile([P, 4], f32, name="e1s", tag="e1s")
            E1S = stt(e1s, e1d, 16.0, s1, op0=ALU.mult, op1=ALU.mult)
            # chunk 3 would land at base partition 96 (invalid: bases must be
            # 0/32/64), so it gets its own base-0 transpose of the last slice;
            # both transposes share one PSUM tile -> a single copy out
            ptp = ptr.tile([P, 2 * P], f32, name="ptp", tag="ptr")
            nc.tensor.transpose(ptp[:, 0:P], v1nP, ident)
            nc.tensor.transpose(ptp[0:32, P:2 * P], v1nP[:, 96:128], ident)
            v1TA = wgt.tile([P, 2 * P], fp16, name="v1TA", tag="v1TA")
            cp(v1TA, ptp)
            xT4h = wgt.tile([P, BC], fp16, name="xT4h", tag="xT4h")
            cp(xT4h, xT4)

        # ======= layer 4 prep (cheap; pinned AFTER the layer-1 DVE chain so
        # it never wedges the s1 path, but early enough to be ready) =======
        if True:
            vt4 = wgt.tile([P, 32], fp16, name="vt4", tag="vt4")
            VT4 = tt(vt4, e4mdT, sm("w4moT"), op=ALU.add)
            dep(E1S, VT4)
            vsq4 = scr.tile([P, 32], fp16, name="vsq4", tag="vsq4")
            mul(vsq4, vt4, vt4)
            n4 = pn.tile([1, 8], f32, name="n4", tag="pn")
            N4MM = []
            for k in range(4):
                N4MM.append(mm(n4, ones4[:, 0:1], vsq4[:, 8 * k:8 * k + 8],
                               start=(k == 0), stop=(k == 3)))
            s4r = make_scale(n4, eg4row, (1, 8), "l4")
            # broadcast s4 to all partitions with a K=1 ones-row matmul
            s4x = pn.tile([P, 32], f32, name="s4x", tag="pn")
            S4X = []
            for k in range(4):
                S4X.append(mm(s4x[:, 8 * k:8 * k + 8], onesr, s4r))
            # fold s4 into both layer-4 stationaries (scale along out-feats)
            vt4s = wgt.tile([P, 32], fp16, name="vt4s", tag="vt4s")
            mul(vt4s, vt4, s4x)
            vd4s = wgt.tile([P, 32], fp16, name="vd4s", tag="vd4s")
            mul(vd4s, e4mdT, s4x)

        # ================= layer 1 batch ==================================
        h1 = wgt.tile([P, 4 * BC], fp16, name="h1", tag="h1")
        G1 = wgt.tile([P, 4 * BC], fp16, name="G1", tag="G1")
        T1 = []
        for c in range(4):
            cs = slice(BC * c, BC * c + BC)
            zc = pz.tile([P, BC], f32, name=f"pz1_{c}", tag=f"pzc{c}")
            if c < 3:
                mm(zc, v1TA[32 * c:32 * c + 8, 0:P], xT4h[32 * c:32 * c + 8, :])
            else:
                mm(zc, v1TA[0:8, P:2 * P], xT4h[0:8, :])
            T1.append(act(h1[:, cs], zc, E.Tanh,
                          bias=sm("b1")[:, c:c + 1], scale=s1[:, c:c + 1]))
            # G1 = (1 - h^2) * e1s: 2 fused DVE ops per chunk
            nh2 = scr.tile([P, BC], fp16, name=f"nh1_{c}", tag=f"nh1_{c}")
            stt(nh2, h1[:, cs], -1.0, h1[:, cs], op0=ALU.mult, op1=ALU.mult)
            ts(G1[:, cs], nh2, 1.0, e1s[:, c:c + 1], op0=ALU.add, op1=ALU.mult)

        # ================= layer 2/3 prep ================================
        def prep_stat(l, vt):
            # in-place exp of the 8 diag 64x64 blocks: two strided ACT ops
            dA = vt[0:64, 0:H].rearrange("p (b c) -> p b c", c=128)[:, :, 0:64]
            a1 = act(dA, dA, E.Exp)
            dB = vt[64:128, 0:H].rearrange("p (b c) -> p b c", c=128)[:, :, 64:128]
            a2 = act(dB, dB, E.Exp)
            # vsq on DVE, split so the window part (gated only by its DMA)
            # can run before the diag part (gated by the exps); both pinned
            # behind the layer-1 DVE chain so they can't wedge it
            vsq = scr.tile([P, 1280], fp16, name=f"vsq{l}", tag="vsq")
            vw = mul(vsq[:, H:1280], vt[:, H:1280], vt[:, H:1280])
            vd = mul(vsq[:, 0:H], vt[:, 0:H], vt[:, 0:H])
            dep(E1S, vw)
            dep(E1S, vd)
            return vsq, a1, a2

        ncol_last = {}
        # one shared psum tile for both layers' norms (cols 0:4 = l2,
        # 4:8 = l3): a single allocation in the "ptr" bank timeline, so no
        # slot-recycling cycles against the pinned ACT chain
        ncol23 = ptr.tile([P, 8], f32, name="ncol23", tag="ptr")

        def prep_norms(l, vsq):
            # norm2 directly in partition layout: vsq-stationary x ones-col
            o = 0 if l == 2 else 4
            with tc.high_priority():
                for c in range(4):
                    for k in range(c + 1):
                        m = mm(ncol23[:, o + c:o + c + 1], _vsl(vsq, k, c),
                               ones4[:, 0:1], start=(k == 0), stop=(k == c))
                ncol_last[l] = m
                return make_scale(ncol23[:, o:o + 4], eg[l], (P, 4), f"l{l}")

        vsq2, A_dA2, A_dB2 = prep_stat(2, vt2)
        s2 = prep_norms(2, vsq2)

        # ================= layer 2/3 batch ================================
        def big_batch(l, vt, s, h_prev, G_prev):
            hl = wgt.tile([P, 4 * BC], fp16, name=f"h{l}", tag=f"h{l}")
            Gl = wgt.tile([P, 4 * BC], fp16, name=f"G{l}", tag=f"G{l}")
            tanhs = []
            for c in range(4):
                cs = slice(BC * c, BC * c + BC)
                zc = pz.tile([P, BC], f32, name=f"pz{l}_{c}", tag=f"pzc{c}")
                for k in range(c + 1):
                    mm(zc, _vsl(vt, k, c), h_prev[:, BC * k:BC * k + BC],
                       start=(k == 0), stop=(k == c))
                tanhs.append(act(hl[:, cs], zc, E.Tanh,
                                 bias=sm(f"b{l}")[:, c:c + 1], scale=s[:, c:c + 1]))
                # pair (c0,c3) and (c1,c2): c3's buffer is freed by c0's
                # early Gin read, so the late G-tail never stalls on PSUM
                fc = pf.tile([P, BC], f32, name=f"pf{l}_{c}",
                             tag="pfpA" if c in (0, 3) else "pfpB")
                F = P * c
                mm(fc[0:64, :], vt[0:64, F:F + 64], G_prev[0:64, cs])
                mm(fc[64:128, :], vt[64:128, F + 64:F + 128], G_prev[64:128, cs])
                # per-chunk G pipeline: sech2 + s-fold, right behind the tanh
                hqc = scr.tile([P, BC], fp16, name=f"hq{l}_{c}", tag=f"hq{l}_{c}")
                mul(hqc, hl[:, cs], hl[:, cs])
                scc = scr.tile([P, BC], fp16, name=f"sc{l}_{c}", tag=f"sc{l}_{c}")
                ts(scc, hqc, -1.0, 1.0, op0=ALU.mult, op1=ALU.add)
                stt(Gl[:, cs], fc, s[:, c:c + 1], scc,
                    op0=ALU.mult, op1=ALU.mult)
            return hl, Gl, tanhs

        h2, G2, T2 = big_batch(2, vt2, s2, h1, G1)

        vsq3, A_dA3, A_dB3 = prep_stat(3, vt3)
        s3 = prep_norms(3, vsq3)

        # keep the cheap-but-LDWEIGHTS-heavy l4-prep matmuls out of the
        # L2->ncol->L3 PE stream: they only feed z4/p4 late in the kernel
        for m4 in N4MM + S4X:
            dep(ncol_last[3], m4)

        h3, G3, T3 = big_batch(3, vt3, s3, h2, G2)

        # ================= layer 4 batch (fully transposed) ===============
        z4 = pn.tile([P, 16], f32, name="z4", tag="pn")
        for b in range(2):
            # bias4 injected via a K=1 ones-row matmul (starts the group)
            mm(z4[:, 8 * b:8 * b + 8], onesr, sm("b4rep")[0:1, 8 * b:8 * b + 8],
               start=True, stop=False, skip_group_check=True)
            for k in range(4):
                mm(z4[:, 8 * b:8 * b + 8],
                   h3[:, BC * k + P * b:BC * k + P * b + P],
                   vt4s[:, 8 * k:8 * k + 8], start=False, stop=(k == 3),
                   skip_group_check=True)
        h4 = wgt.tile([P, 16], f32, name="h4", tag="h4")
        T4 = act(h4, z4, E.Tanh)
        nc.sync.dma_start(t["h4T_out"], h4)
        p4 = ptr.tile([P, 16], f32, name="p4", tag="ptr")
        for b in range(2):
            for k in range(4):
                mm(p4[:, 8 * b:8 * b + 8],
                   G3[:, BC * k + P * b:BC * k + P * b + P],
                   vd4s[:, 8 * k:8 * k + 8], start=(k == 0), stop=(k == 3))
        with tc.high_priority():
            # gt = p4 * (1 - h4^2) in two fused stt ops; s4 already in vd4s
            nh4 = scr.tile([P, 16], f32, name="nh4", tag="nh4")
            stt(nh4, h4, -1.0, h4, op0=ALU.mult, op1=ALU.mult)
            gt = wgt.tile([P, 16], f32, name="gt", tag="gt")
            stt(gt, nh4, 1.0, p4, op0=ALU.add, op1=ALU.mult)
            # fast log: ln(x) ~= LN2_A * float(bits(x)) + LN2_B
            gf = scr.tile([P, 16], f32, name="gf", tag="gf")
            cp(gf, gt.bitcast(u32))
            sld = wgt.tile([P, 16], f32, name="sld", tag="sld")
            ts(sld, gf, LN2_A, LN2_B, op0=ALU.mult, op1=ALU.add)
        nc.sync.dma_start(t["sldT_out"], sld)

        # ---- pin the ACT queue order: interleave the l2/l3 prep exps with
        # the tanhs so neither starves the other ----
        chain = ([A_esm, A_dA2, T1[0], A_dB2, T1[1], T1[2], T1[3],
                  T2[0], A_dA3, A_dB3, T2[1], T2[2], T2[3]]
                 + T3 + [T4])
        for a, b in zip(chain, chain[1:]):
            dep(a, b)

    nc.compile()
    return nc


def _host_prep(x, W1, logg1, bias1, W2, logg2, bias2, W3, logg3, bias3,
               W4, logg4, bias4):
    """Pure layout prep (transpose / reshape / gather / masks), no arithmetic."""
    f = np.float32

    def cols(a):          # [512]-ish vector -> [128, 4] column-chunk layout
        return np.ascontiguousarray(np.reshape(a, (4, P)).T).astype(f)

    def fold(m):          # [512, 8] -> [128, (k x)] with k = row-chunk
        return m.reshape(4, P, 8).transpose(1, 0, 2).reshape(P, 32)

    def pack(WT):         # [512, 512] W.T -> [128, 1280] diag strip + windows
        wp = np.empty((P, 1280), f)
        for k in range(4):
            d = np.array(WT[P * k:P * k + P, P * k:P * k + P])
            d[64:128, 0:64] = 0.0          # structural mask: LL quadrant
            wp[:, P * k:P * k + P] = d
        for k in range(3):
            w = 384 - 128 * k
            wp[:, OFF2[k]:OFF2[k] + w] = WT[P * k:P * k + P, P * (k + 1):H]
        return wp

    smalls = np.zeros((P, SMALL_W), f)

    def put(name, arr):
        a, b = _SM[name]
        smalls[:arr.shape[0], a:b] = arr

    put("w1dg", cols(W1[np.arange(H), np.arange(H) // 64]))
    put("lg1", cols(logg1)); put("b1", cols(bias1))
    put("lg2", cols(logg2)); put("b2", cols(bias2))
    put("lg3", cols(logg3)); put("b3", cols(bias3))
    smalls[0, _SM["lg4r"][0]:_SM["lg4r"][0] + 8] = np.asarray(logg4).reshape(8)
    put("b4rep", np.concatenate([np.asarray(bias4).reshape(8)] * 2).reshape(1, 16))
    # structural masks (pre-applied on the host; -100 marks exp->0 positions)
    o = np.arange(H)[:, None] // 64
    i1 = np.arange(8)[None, :]
    md1 = (i1 == o); mo1 = (i1 < o)                        # [512, 8] natural
    W1n = np.asarray(W1).astype(f)
    put("w1mdN", fold(np.where(md1, W1n, f(-100.0))))
    put("w1moN", fold(np.where(mo1, W1n, f(0.0))))
    W4T = np.ascontiguousarray(np.asarray(W4).T).astype(f)  # [512, 8]
    ii = np.arange(H)[:, None] // 64
    o4 = np.arange(8)[None, :]
    md4 = (o4 == ii); mo4 = (o4 > ii)
    put("w4mdT", fold(np.where(md4, W4T, f(-100.0))))
    put("w4moT", fold(np.where(mo4, W4T, f(0.0))))

    wp2 = pack(np.ascontiguousarray(np.asarray(W2).T).astype(f))
    wp3 = pack(np.ascontiguousarray(np.asarray(W3).T).astype(f))
    xT = np.ascontiguousarray(np.asarray(x).T).astype(f)   # [8, 2048]
    return xT, wp2, wp3, smalls


def kernel(**inputs):
    global LAST_RESULTS
    from concourse.bass_utils import run_bass_kernel_spmd

    xT, wp2, wp3, smalls = _host_prep(**{k: np.asarray(v) for k, v in inputs.items()})

    if "nc" not in _CACHE:
        _CACHE["nc"] = _build()
    nc = _CACHE["nc"]

    wp2d = np.ascontiguousarray(wp2[:, 0:512]); wp2w = np.ascontiguousarray(wp2[:, 512:1280])
    wp3d = np.ascontiguousarray(wp3[:, 0:512]); wp3w = np.ascontiguousarray(wp3[:, 512:1280])
    in_maps = []
    for c in range(NCORE):
        # x.T row-replicated 16x so rows 32c+i == x.T[i] for the 32-aligned
        # L1 stationary slices
        xt4 = np.ascontiguousarray(np.tile(xT[:, BC * c:BC * (c + 1)], (16, 1)))
        in_maps.append({
            "xT4": xt4, "wp2d": wp2d, "wp2w": wp2w, "wp3d": wp3d,
            "wp3w": wp3w, "smalls": smalls,
        })
    res = run_bass_kernel_spmd(nc, in_maps, core_ids=list(range(NCORE)),
                               trace=TRACE)
    LAST_RESULTS = res

    B = BC * NCORE
    h = np.empty((B, 8), np.float32)
    sld = np.empty((B, 8), np.float32)
    for c, r in enumerate(res.results):
        h4 = r["h4T_out"]          # [128, 16]: h[128b+p, o] = h4[p, 8b+o]
        sl = r["sldT_out"]
        for b in range(2):
            h[BC * c + P * b: BC * c + P * (b + 1)] = h4[:, 8 * b:8 * b + 8]
            sld[BC * c + P * b: BC * c + P * (b + 1)] = sl[:, 8 * b:8 * b + 8]
    return h, sld


# revision 58
# speedup vs baseline: 1.1177x; 1.0467x over previous
"""BNAF forward (B=2048, D=8, H=512, 4 masked layers) on 8 TRN2 NeuronCores.

Strategy
--------
Pure data parallel: batch is split 256/core; the small weights are replicated.

Math: the BNAF log-det recursion collapses in exp space.  For each masked
linear layer, exp(logdet diag blocks) == the diag blocks of the normalized
weight w itself, and for tanh, exp(logdet) == 1 - h^2.  So the whole
log-sum-exp flow is a chain of *positive* block-diagonal matmuls with one
log() at the very end.  The per-output norm scale s = exp(logg)/||v|| is
folded into the G-flow moving operand (G_in = s * G), so the G-flow
stationary is just exp(W) block-diag.

Final layout/schedule:
- weights land as packed SWDGE fp32->fp16 cast-DMAs: per layer a diag-strip
  DMA ([[Wd_A, UR],[0, Wd_B]] per chunk, host-zeroed LL) then the
  strictly-lower windows, so the diag exps start ~2us before the windows.
- masked weights for layers 1/4 use the host "-100 trick":
  exp(where(mask_d, W, -100)) == exp(W)*mask_d, so building the masked
  stationaries is a single DVE add each (no mask multiplies, no PE
  transposes: the layer-1 stationary is exp'd directly in [8,512] layout).
- diag exp is TWO in-place strided ACT ops per layer (dA strip, dB strip);
  the G-flow runs on the same exp'd diag via 64x64 tile_position matmuls
  (no separate block-diag stationary); G tiles are fp16 scaled by 2^4
  (folded out of the final log constant) to stay in normal range.
- per-out-feature norms: vsq (ACT Square, split diag/windows) then
  10 accumulating vsq-stationary x ones-column matmuls -> [128,4] psum
  directly in partition layout -> short DVE Newton rsqrt (magic + 1 step).
- per-chunk PSUM tiles everywhere: region-independent tiles keep the
  h-matmul stream free of WAR coupling to the previous chunk's tanh.
- layer 4 runs fully transposed (stationary = h3/G3 batch-halves, moving =
  the tiny [128,8] layer-4 weights) producing [128,16] tiles: batch on
  partitions, so the tail elementwise chain + final log run on 128 lanes.
- final Ln is a 2-op DVE fast-log (bitcast + affine), no second ACT table.
- ACT only ever uses {Exp, Tanh, Square}: single table load at start.
"""

import numpy as np

TRACE = False          # set by test.py for profiling runs
LAST_RESULTS = None    # BassKernelResults stash for test.py

_CACHE = {}

P = 128
BC = 256          # batch per core
H = 512
NCORE = 8
MAGIC = 0x5f3759df
OFF2 = (512, 896, 1152)   # packed col offsets of the strictly-lower windows
LN2_A = 8.262958294867817e-08     # ln2 * 2^-23
LN2_B = -90.77247532458875        # -126.9570 * ln2 - 4*ln2 (G-flow 2^4 scale)

# smalls layout: first the exp block (exp'd in one ACT op), then the rest.
# wNmd* entries hold where(mask_d, W, -100): exp gives exp(W)*mask_d exactly,
# so the masked-linear weights need no on-device mask multiplies.
_SM = {}
_off = 0
for _name, _w in [("w1dg", 4), ("lg1", 4), ("lg2", 4), ("lg3", 4),
                  ("lg4r", 8), ("w1mdN", 32), ("w4mdT", 32),      # exp block
                  ("b1", 4), ("b2", 4), ("b3", 4), ("b4rep", 16),
                  ("w1moN", 32), ("w4moT", 32)]:
    _SM[_name] = (_off, _off + _w)
    _off += _w
SMALL_W = _off
EXPW = _SM["w4mdT"][1]        # width of the exp block (88)


def _vsl(vt, k, c):
    """Packed-layout slice of the (in-chunk k, out-chunk c) 128x128 block."""
    if k == c:
        return vt[:, 128 * k:128 * k + 128]
    o = OFF2[k] + 128 * (c - k - 1)
    return vt[:, o:o + 128]


def _build():
    import concourse.bacc as bacc
    import concourse.mybir as mybir
    import concourse.tile as tile
    from concourse.tile_rust import add_dep_helper
    from contextlib import ExitStack

    f32 = mybir.dt.float32
    u32 = mybir.dt.uint32
    bf16 = mybir.dt.bfloat16
    fp16 = mybir.dt.float16
    E = mybir.ActivationFunctionType
    ALU = mybir.AluOpType

    nc = bacc.Bacc("TRN2", target_bir_lowering=False, debug=False,
                   enable_asserts=False, num_devices=NCORE)

    t = {}
    t["xT"] = nc.dram_tensor("xT", (8, BC), f32, kind="ExternalInput").ap()
    t["wp2d"] = nc.dram_tensor("wp2d", (P, H), f32, kind="ExternalInput").ap()
    t["wp2w"] = nc.dram_tensor("wp2w", (P, 768), f32, kind="ExternalInput").ap()
    t["wp3d"] = nc.dram_tensor("wp3d", (P, H), f32, kind="ExternalInput").ap()
    t["wp3w"] = nc.dram_tensor("wp3w", (P, 768), f32, kind="ExternalInput").ap()
    t["w1s"] = nc.dram_tensor("w1s", (8, 1024), f32, kind="ExternalInput").ap()
    t["smalls"] = nc.dram_tensor("smalls", (P, SMALL_W), f32, kind="ExternalInput").ap()
    t["h4T_out"] = nc.dram_tensor("h4T_out", (P, 16), f32, kind="ExternalOutput").ap()
    t["sldT_out"] = nc.dram_tensor("sldT_out", (P, 16), f32, kind="ExternalOutput").ap()

    def mm(out, lhsT, rhs, **kw):
        return nc.tensor.matmul(out, lhsT, rhs, **kw)

    def dep(a, b):
        """Pin engine-queue order: instruction a runs before b (no sem)."""
        add_dep_helper(b.ins, a.ins, False, "order-pin")

    with tile.TileContext(nc) as tc, ExitStack() as ctx:
        wgt = ctx.enter_context(tc.tile_pool(name="wgt", bufs=1))
        scr = ctx.enter_context(tc.tile_pool(name="scr", bufs=3))
        pz = ctx.enter_context(tc.tile_pool(name="pz", bufs=1, space="PSUM"))
        pf = ctx.enter_context(tc.tile_pool(name="pf", bufs=1, space="PSUM"))
        pn = ctx.enter_context(tc.tile_pool(name="pn", bufs=1, space="PSUM"))
        pq = ctx.enter_context(tc.tile_pool(name="pq", bufs=1, space="PSUM"))

        act = nc.scalar.activation
        cp = nc.vector.tensor_copy
        ts = nc.vector.tensor_scalar
        stt = nc.vector.scalar_tensor_tensor
        mul = nc.vector.tensor_mul
        tt = nc.vector.tensor_tensor

        # ---- input DMAs: smalls on HWDGE; x + packed weights as SWDGE
        # cast-DMAs (fp32 DRAM -> fp16 SBUF, cast inline in the SDMA) ----
        smalls = wgt.tile([P, SMALL_W], f32, name="smalls_t", tag="smalls_t")
        nc.sync.dma_start(smalls, t["smalls"])
        w1s = wgt.tile([8, 1024], f32, name="w1s_t", tag="w1s_t")
        nc.scalar.dma_start(w1s, t["w1s"])
        xTt = wgt.tile([8, BC], fp16, name="xTt", tag="xTt")
        nc.gpsimd.dma_start(xTt, t["xT"])
        vt2 = wgt.tile([P, 1280], fp16, name="vt2", tag="vt2")
        nc.gpsimd.dma_start(vt2[:, 0:H], t["wp2d"])
        nc.gpsimd.dma_start(vt2[:, H:1280], t["wp2w"])
        vt3 = wgt.tile([P, 1280], fp16, name="vt3", tag="vt3")
        nc.gpsimd.dma_start(vt3[:, 0:H], t["wp3d"])
        nc.gpsimd.dma_start(vt3[:, H:1280], t["wp3w"])

        def sm(name):
            a, b = _SM[name]
            return smalls[:, a:b]

        # ---- tiny constants on DVE (keep Q7 free) ----
        magict = wgt.tile([P, 8], u32, name="magict", tag="magict")
        nc.vector.memset(magict, MAGIC)
        ones4f = wgt.tile([P, 4], f32, name="ones4f", tag="ones4f")
        nc.vector.memset(ones4f, 1.0)
        ones4 = wgt.tile([P, 4], fp16, name="ones4", tag="ones4")
        cp(ones4, ones4f)
        wz = wgt.tile([P, BC], fp16, name="wz", tag="wz")
        nc.vector.memset(wz, 0.0)
        onesr = wgt.tile([1, P], f32, name="onesr", tag="onesr")
        nc.vector.memset(onesr, 1.0)

        # short PE warm-up burst (HAM un-throttle) while DMAs drain
        pw = pn.tile([2, BC - 2], f32, name="pw", tag="pn")
        for _ in range(14):
            mm(pw, wz[:, 0:2], wz[:, 2:BC], skip_group_check=True)

        # one batched exp over the whole exp block
        esm = wgt.tile([P, EXPW], f32, name="esm", tag="esm")
        A_esm = act(esm, smalls[:, 0:EXPW], E.Exp)

        def esl(name):
            a, b = _SM[name]
            return esm[:, a:b]

        e1d = esl("w1dg")
        eg = {1: esl("lg1"), 2: esl("lg2"), 3: esl("lg3")}
        eg4row = esm[0:1, _SM["lg4r"][0]:_SM["lg4r"][0] + 8]
        e1mdN = esl("w1mdN")
        e4mdT = esl("w4mdT")
        # exp of the masked-diag W1.T row block [8, 512]
        e1T = wgt.tile([8, H], f32, name="e1T", tag="e1T")
        A_e1T = act(e1T, w1s[:, 0:H], E.Exp)

        # s = eg * rsqrt(norm2): DVE-only Newton rsqrt (reads n2 psum directly)
        def make_scale(n2_ap, eg_ap, shape, nm):
            pr = shape[0]
            shf = scr.tile(list(shape), u32, name=f"shf_{nm}", tag="sc_shf")
            ts(shf, n2_ap.bitcast(u32), 1, None, op0=ALU.arith_shift_right)
            y0 = scr.tile(list(shape), u32, name=f"y0_{nm}", tag="sc_y0")
            stt(y0, magict[:pr, :shape[1]], 0, shf, op0=ALU.bypass, op1=ALU.subtract)
            y = y0.bitcast(f32)
            t1 = scr.tile(list(shape), f32, name=f"t1_{nm}", tag="sc_t1")
            t2 = scr.tile(list(shape), f32, name=f"t2_{nm}", tag="sc_t2")
            for it in range(1):         # one Newton step: y *= 1.5 - 0.5*n2*y*y
                mul(t1, y, y)
                mul(t2, t1, n2_ap)
                ts(t1, t2, -0.5, 1.5, op0=ALU.mult, op1=ALU.add)
                yn = scr.tile(list(shape), f32, name=f"yn{it}_{nm}", tag=f"sc_yn{it}")
                mul(yn, y, t1)
                y = yn
            s = wgt.tile(list(shape), f32, name=f"s_{nm}", tag=f"s_{nm}")
            mul(s, eg_ap, y)
            return s

        # ================= layer 1 prep ===================================
        # v1n (natural fold, for norms) and v1T (stationary): one add each.
        # High priority: s1 gates the whole h/G cascade.
        with tc.high_priority():
            v1n = wgt.tile([P, 32], f32, name="v1n", tag="v1n")
            tt(v1n, e1mdN, sm("w1moN"), op=ALU.add)
            n1 = wgt.tile([P, 4], f32, name="n1", tag="n1")
            for c in range(4):
                sq1 = scr.tile([P, 8], f32, name=f"sq1_{c}", tag="sq1")
                stt(sq1, v1n[:, 8 * c:8 * c + 8], 0, v1n[:, 8 * c:8 * c + 8],
                    op0=ALU.bypass, op1=ALU.mult, accum_out=n1[:, c:c + 1])
            s1 = make_scale(n1, eg[1], (P, 4), "l1")
            e1s = wgt.tile([P, 4], f32, name="e1s", tag="e1s")
            stt(e1s, e1d, 16.0, s1, op0=ALU.mult, op1=ALU.mult)
            v1T = wgt.tile([8, H], fp16, name="v1T", tag="v1T")
            tt(v1T, e1T, w1s[:, H:2 * H], op=ALU.add)

        # ======= layer 4 prep (cheap; all inputs ready early, so keep the
        # whole chain high-priority to stop the scheduler drifting it late) ==
        with tc.high_priority():
            vt4 = wgt.tile([P, 32], fp16, name="vt4", tag="vt4")
            tt(vt4, e4mdT, sm("w4moT"), op=ALU.add)
            vsq4 = scr.tile([P, 32], fp16, name="vsq4", tag="vsq4")
            mul(vsq4, vt4, vt4)
            n4 = pn.tile([1, 8], f32, name="n4", tag="pn")
            for k in range(4):
                mm(n4, ones4[:, 0:1], vsq4[:, 8 * k:8 * k + 8],
                   start=(k == 0), stop=(k == 3))
            s4r = make_scale(n4, eg4row, (1, 8), "l4")
            # broadcast s4 to all partitions with a K=1 ones-row matmul
            s4x = pn.tile([P, 32], f32, name="s4x", tag="pn")
            for k in range(4):
                mm(s4x[:, 8 * k:8 * k + 8], onesr, s4r)
            # fold s4 into both layer-4 stationaries (scale along out-feats)
            vt4s = wgt.tile([P, 32], fp16, name="vt4s", tag="vt4s")
            mul(vt4s, vt4, s4x)
            vd4s = wgt.tile([P, 32], fp16, name="vd4s", tag="vd4s")
            mul(vd4s, e4mdT, s4x)

        # ================= layer 1 batch ==================================
        h1 = wgt.tile([P, 4 * BC], fp16, name="h1", tag="h1")
        G1 = wgt.tile([P, 4 * BC], fp16, name="G1", tag="G1")
        T1 = []
        for c in range(4):
            cs = slice(BC * c, BC * c + BC)
            zc = pz.tile([P, BC], f32, name=f"pz1_{c}", tag=f"pzc{c}")
            mm(zc, v1T[:, P * c:P * c + P], xTt)
            T1.append(act(h1[:, cs], zc, E.Tanh,
                          bias=sm("b1")[:, c:c + 1], scale=s1[:, c:c + 1]))
            hqc = scr.tile([P, BC], fp16, name=f"hq1_{c}", tag="hqc")
            mul(hqc, h1[:, cs], h1[:, cs])
            scc = scr.tile([P, BC], fp16, name=f"sc1_{c}", tag="scc")
            ts(scc, hqc, -1.0, 1.0, op0=ALU.mult, op1=ALU.add)
            ts(G1[:, cs], scc, e1s[:, c:c + 1], None, op0=ALU.mult)

        # ================= layer 2/3 prep ================================
        def prep_stat(l, vt):
            # in-place exp of the 8 diag 64x64 blocks: two strided ACT ops
            dA = vt[0:64, 0:H].rearrange("p (b c) -> p b c", c=128)[:, :, 0:64]
            a1 = act(dA, dA, E.Exp)
            dB = vt[64:128, 0:H].rearrange("p (b c) -> p b c", c=128)[:, :, 64:128]
            a2 = act(dB, dB, E.Exp)
            # vsq split: diag strip right after the exps, windows on arrival;
            # on ACT (Square) so it can't wedge the DVE rsqrt chains
            vsq = scr.tile([P, 1280], fp16, name=f"vsq{l}", tag="vsq")
            q1 = act(vsq[:, 0:H], vt[:, 0:H], E.Square)
            q2 = act(vsq[:, H:1280], vt[:, H:1280], E.Square)
            return vsq, a1, a2, q1, q2

        def prep_norms(l, vsq):
            # norm2 directly in partition layout: vsq-stationary x ones-col
            ncol = pn.tile([P, 4], f32, name=f"ncol{l}", tag="pn")
            with tc.high_priority():
                for c in range(4):
                    for k in range(c + 1):
                        mm(ncol[:, c:c + 1], _vsl(vsq, k, c), ones4[:, 0:1],
                           start=(k == 0), stop=(k == c))
                return make_scale(ncol, eg[l], (P, 4), f"l{l}")

        vsq2, A_dA2, A_dB2, Q2d, Q2w = prep_stat(2, vt2)
        s2 = prep_norms(2, vsq2)
        vsq3, A_dA3, A_dB3, Q3d, Q3w = prep_stat(3, vt3)

        # ================= layer 2/3 batch ================================
        def big_batch(l, vt, s, h_prev, G_prev):
            hl = wgt.tile([P, 4 * BC], fp16, name=f"h{l}", tag=f"h{l}")
            Gl = wgt.tile([P, 4 * BC], fp16, name=f"G{l}", tag=f"G{l}")
            tanhs = []
            for c in range(4):
                cs = slice(BC * c, BC * c + BC)
                zc = pz.tile([P, BC], f32, name=f"pz{l}_{c}", tag=f"pzc{c}")
                for k in range(c + 1):
                    mm(zc, _vsl(vt, k, c), h_prev[:, BC * k:BC * k + BC],
                       start=(k == 0), stop=(k == c))
                tanhs.append(act(hl[:, cs], zc, E.Tanh,
                                 bias=sm(f"b{l}")[:, c:c + 1], scale=s[:, c:c + 1]))
                # pair (c0,c3) and (c1,c2): c3's buffer is freed by c0's
                # early Gin read, so the late G-tail never stalls on PSUM
                fc = pf.tile([P, BC], f32, name=f"pf{l}_{c}",
                             tag="pfpA" if c in (0, 3) else "pfpB")
                F = P * c
                mm(fc[0:64, :], vt[0:64, F:F + 64], G_prev[0:64, cs])
                mm(fc[64:128, :], vt[64:128, F + 64:F + 128], G_prev[64:128, cs])
                # per-chunk G pipeline: sech2 + s-fold, right behind the tanh
                hqc = scr.tile([P, BC], fp16, name=f"hq{l}_{c}", tag="hqc")
                mul(hqc, hl[:, cs], hl[:, cs])
                scc = scr.tile([P, BC], fp16, name=f"sc{l}_{c}", tag="scc")
                ts(scc, hqc, -1.0, 1.0, op0=ALU.mult, op1=ALU.add)
                stt(Gl[:, cs], fc, s[:, c:c + 1], scc,
                    op0=ALU.mult, op1=ALU.mult)
            return hl, Gl, tanhs

        h2, G2, T2 = big_batch(2, vt2, s2, h1, G1)

        s3 = prep_norms(3, vsq3)

        h3, G3, T3 = big_batch(3, vt3, s3, h2, G2)

        # ================= layer 4 batch (fully transposed) ===============
        z4 = pn.tile([P, 16], f32, name="z4", tag="pn")
        for b in range(2):
            for k in range(4):
                mm(z4[:, 8 * b:8 * b + 8],
                   h3[:, BC * k + P * b:BC * k + P * b + P],
                   vt4s[:, 8 * k:8 * k + 8], start=(k == 0), stop=(k == 3))
        # s4 is already in vt4s; just add the bias in transposed layout
        with tc.high_priority():
            z4t = scr.tile([P, 16], f32, name="z4t", tag="z4t")
            tt(z4t, z4, sm("b4rep"), op=ALU.add)
        h4 = wgt.tile([P, 16], f32, name="h4", tag="h4")
        T4 = act(h4, z4t, E.Tanh)
        nc.sync.dma_start(t["h4T_out"], h4)
        p4 = pq.tile([P, 16], f32, name="p4", tag="pq")
        for b in range(2):
            for k in range(4):
                mm(p4[:, 8 * b:8 * b + 8],
                   G3[:, BC * k + P * b:BC * k + P * b + P],
                   vd4s[:, 8 * k:8 * k + 8], start=(k == 0), stop=(k == 3))
        with tc.high_priority():
            # gt = p4 * (1 - h4^2) in two fused stt ops; s4 already in vd4s
            hq4 = scr.tile([P, 16], f32, name="hq4", tag="hq4")
            stt(hq4, h4, -1.0, h4, op0=ALU.mult, op1=ALU.mult)
            gt = wgt.tile([P, 16], f32, name="gt", tag="gt")
            stt(gt, hq4, 1.0, p4, op0=ALU.add, op1=ALU.mult)
            # fast log: ln(x) ~= LN2_A * float(bits(x)) + LN2_B
            gf = scr.tile([P, 16], f32, name="gf", tag="gf")
            cp(gf, gt.bitcast(u32))
            sld = wgt.tile([P, 16], f32, name="sld", tag="sld")
            ts(sld, gf, LN2_A, LN2_B, op0=ALU.mult, op1=ALU.add)
        nc.sync.dma_start(t["sldT_out"], sld)

        # ---- pin the ACT queue order: tanhs are never stuck behind the
        # next layer's prep (exps/squares), which the scheduler's DMA-latency
        # model otherwise misplaces; l3 squares interleave into T2 ----
        chain = ([A_esm, A_e1T, A_dA2, A_dB2] + T1 +
                 [Q2d, Q2w, A_dA3, A_dB3,
                  T2[0], T2[1], Q3d, T2[2], Q3w, T2[3]] + T3 + [T4])
        for a, b in zip(chain, chain[1:]):
            dep(a, b)

    nc.compile()
    return nc


def _host_prep(x, W1, logg1, bias1, W2, logg2, bias2, W3, logg3, bias3,
               W4, logg4, bias4):
    """Pure layout prep (transpose / reshape / gather / masks), no arithmetic."""
    f = np.float32

    def cols(a):          # [512]-ish vector -> [128, 4] column-chunk layout
        return np.ascontiguousarray(np.reshape(a, (4, P)).T).astype(f)

    def fold(m):          # [512, 8] -> [128, (k x)] with k = row-chunk
        return m.reshape(4, P, 8).transpose(1, 0, 2).reshape(P, 32)

    def pack(WT):         # [512, 512] W.T -> [128, 1280] diag strip + windows
        wp = np.empty((P, 1280), f)
        for k in range(4):
            d = np.array(WT[P * k:P * k + P, P * k:P * k + P])
            d[64:128, 0:64] = 0.0          # structural mask: LL quadrant
            wp[:, P * k:P * k + P] = d
        for k in range(3):
            w = 384 - 128 * k
            wp[:, OFF2[k]:OFF2[k] + w] = WT[P * k:P * k + P, P * (k + 1):H]
        return wp

    smalls = np.zeros((P, SMALL_W), f)

    def put(name, arr):
        a, b = _SM[name]
        smalls[:arr.shape[0], a:b] = arr

    put("w1dg", cols(W1[np.arange(H), np.arange(H) // 64]))
    put("lg1", cols(logg1)); put("b1", cols(bias1))
    put("lg2", cols(logg2)); put("b2", cols(bias2))
    put("lg3", cols(logg3)); put("b3", cols(bias3))
    smalls[0, _SM["lg4r"][0]:_SM["lg4r"][0] + 8] = np.asarray(logg4).reshape(8)
    put("b4rep", np.broadcast_to(
        np.concatenate([np.asarray(bias4).reshape(8)] * 2).reshape(1, 16),
        (P, 16)))
    # structural masks (pre-applied on the host; -100 marks exp->0 positions)
    o = np.arange(H)[:, None] // 64
    i1 = np.arange(8)[None, :]
    md1 = (i1 == o); mo1 = (i1 < o)                        # [512, 8] natural
    W1n = np.asarray(W1).astype(f)
    put("w1mdN", fold(np.where(md1, W1n, f(-100.0))))
    put("w1moN", fold(np.where(mo1, W1n, f(0.0))))
    W4T = np.ascontiguousarray(np.asarray(W4).T).astype(f)  # [512, 8]
    ii = np.arange(H)[:, None] // 64
    o4 = np.arange(8)[None, :]
    md4 = (o4 == ii); mo4 = (o4 > ii)
    put("w4mdT", fold(np.where(md4, W4T, f(-100.0))))
    put("w4moT", fold(np.where(mo4, W4T, f(0.0))))
    # [8, 1024] row-block: [ where(md1.T, W1.T, -100) | where(mo1.T, W1.T, 0) ]
    W1T = np.ascontiguousarray(W1n.T)                      # [8, 512]
    w1s = np.concatenate([np.where(md1.T, W1T, f(-100.0)),
                          np.where(mo1.T, W1T, f(0.0))], axis=1).astype(f)

    wp2 = pack(np.ascontiguousarray(np.asarray(W2).T).astype(f))
    wp3 = pack(np.ascontiguousarray(np.asarray(W3).T).astype(f))
    xT = np.ascontiguousarray(np.asarray(x).T).astype(f)   # [8, 2048]
    return xT, wp2, wp3, smalls, w1s


def kernel(**inputs):
    global LAST_RESULTS
    from concourse.bass_utils import run_bass_kernel_spmd

    xT, wp2, wp3, smalls, w1s = _host_prep(**{k: np.asarray(v) for k, v in inputs.items()})

    if "nc" not in _CACHE:
        _CACHE["nc"] = _build()
    nc = _CACHE["nc"]

    wp2d = np.ascontiguousarray(wp2[:, 0:512]); wp2w = np.ascontiguousarray(wp2[:, 512:1280])
    wp3d = np.ascontiguousarray(wp3[:, 0:512]); wp3w = np.ascontiguousarray(wp3[:, 512:1280])
    in_maps = []
    for c in range(NCORE):
        in_maps.append({
            "xT": np.ascontiguousarray(xT[:, BC * c:BC * (c + 1)]),
            "wp2d": wp2d, "wp2w": wp2w, "wp3d": wp3d, "wp3w": wp3w,
            "smalls": smalls, "w1s": w1s,
        })
    res = run_bass_kernel_spmd(nc, in_maps, core_ids=list(range(NCORE)),
                               trace=TRACE)
    LAST_RESULTS = res

    B = BC * NCORE
    h = np.empty((B, 8), np.float32)
    sld = np.empty((B, 8), np.float32)
    for c, r in enumerate(res.results):
        h4 = r["h4T_out"]          # [128, 16]: h[128b+p, o] = h4[p, 8b+o]
        sl = r["sldT_out"]
        for b in range(2):
            h[BC * c + P * b: BC * c + P * (b + 1)] = h4[:, 8 * b:8 * b + 8]
            sld[BC * c + P * b: BC * c + P * (b + 1)] = sl[:, 8 * b:8 * b + 8]
    return h, sld

